# revision 1
# baseline (speedup 1.0000x reference)
"""BiLevelRoutingAttention Trainium2 kernel (8-core SPMD).

Sharding: core r handles batch b = r//4 and windows w in [ (r%4)*8, (r%4)*8+8 ).
Routing (region top-k) is computed on host via linearity of the mean:
    q_region = mean_{t,s}(xw) @ Wq + bq  (exact up to fp reassociation).
Spikes are binary -> all attention arithmetic is exact integer math in fp16
(counts <= 2048 are exactly representable). Cross-window kv sums need R
(per-region k^T v) from sibling cores -> AllGather over each batch's 4-core
group.

QKV matmul precision (KQKV env): "f16x3" (default; x,w split into fp16 hi+lo,
3 passes, ~1e-4 rel err), "float32r" (1 pass, ~1e-3), "float32" (exact).
Proj precision (KPJ): "float32r" (default) or "float32".
"""

import numpy as np
import ml_dtypes
import os as _os

# ---- problem constants (hardcoded per contract) ----
T, B, Lt, Lh, Lw, C = 4, 2, 8, 32, 32, 256
WT, WH, WW = 2, 4, 4
W = WT * WH * WW            # 32 windows
GT, GH, GW = Lt // WT, Lh // WH, Lw // WW
S = GT * GH * GW            # 256 tokens per window
H, D = 8, C // 8            # 8 heads, 32 dim
TOPK = 8
SCALE = float(D) ** -0.5
NCORES = 8
NW = 8                      # windows per core
NTOK = T * S                # 1024 token-instances per window

QKV_MODE = _os.environ.get("KQKV", "f16x3")     # f16x3 | float32r | float32
PJ_DT_NAME = _os.environ.get("KPJ", "float32r")  # float32r | float32
# NOTE: DVE tensor_scalar is_ge with fp16 output is numerically wrong on HW
# (2.6e-3 rel err vs 3.1e-6); gpsimd is exact. Keep spikes on gpsimd.
GE_ENG = _os.environ.get("KGE", "gpsimd")

_prog_cache = {}


def _dedup_ldweights(nc, mybir):
    """Drop an InstLdweights whose stationary operand is byte-identical to the
    immediately preceding PE Ldweights with only Matmults in between (the PE
    weight slot still holds the same data). Waits/updates are folded into the
    next PE instruction."""
    ndrop = 0
    for bb in nc.main_func.blocks:
        new_list = []
        last_sig = None
        pending = None   # (waits, updates) from a dropped ldw
        for ins in bb.instructions:
            tn = type(ins).__name__
            if ins.engine != mybir.EngineType.PE:
                new_list.append(ins)
                continue
            if pending is not None and tn in ("InstLdweights", "InstMatmult"):
                si = ins.sync_info
                w = list(si.on_wait) if si and si.on_wait else []
                u = list(si.on_update) if si and si.on_update else []
                ins.sync_info = mybir.SyncInfo(on_wait=pending[0] + w,
                                               on_update=pending[1] + u)
                pending = None
            if tn == "InstLdweights":
                try:
                    ap = ins.ins[0]
                    sig = repr(ap)
                except Exception:
                    sig = None
                if sig is not None and sig == last_sig:
                    si = ins.sync_info
                    w = list(si.on_wait) if si and si.on_wait else []
                    u = list(si.on_update) if si and si.on_update else []
                    pending = (w, u)
                    ndrop += 1
                    continue
                last_sig = sig
            elif tn != "InstMatmult":
                last_sig = None
            new_list.append(ins)
        assert pending is None or not (pending[0] or pending[1])
        bb.instructions[:] = new_list
    return ndrop


def _split_sync_waits(nc, mybir, maxw=1):
    """walrus in this container rejects >1 sync wait per instruction; split
    excess waits onto NoOp instructions inserted just before."""
    for bb in nc.main_func.blocks:
        new_list = []
        for ins in bb.instructions:
            si = ins.sync_info
            waits = list(si.on_wait) if si is not None and si.on_wait else []
            if len(waits) > maxw:
                extra = waits[:-maxw]
                keep = waits[-maxw:]
                idx = 0
                while extra:
                    chunk, extra = extra[:maxw], extra[maxw:]
                    nop = mybir.InstNoOp(name=f"{ins.name}-wsplit{idx}", ins=[], outs=[])
                    nop.engine = ins.engine
                    nop.sync_info = mybir.SyncInfo(on_wait=chunk, on_update=[])
                    new_list.append(nop)
                    idx += 1
                ins.sync_info = mybir.SyncInfo(
                    on_wait=keep,
                    on_update=list(si.on_update) if si.on_update else [],
                )
            new_list.append(ins)
        bb.instructions[:] = new_list


def _build_program(routing_idx, sim_mode=False):
    """routing_idx: [B, W, TOPK] int array (host-computed). Returns nc.
    sim_mode: no collective / no If-chain (single-core TimelineSim)."""
    import contextlib
    import concourse.bass as bass
    import concourse.mybir as mybir
    import concourse.tile as tile

    f32 = mybir.dt.float32
    f16 = mybir.dt.float16
    bf16 = mybir.dt.bfloat16
    pj_dt = getattr(mybir.dt, PJ_DT_NAME)
    split3 = QKV_MODE == "f16x3"
    qkv_dt = f16 if split3 else getattr(mybir.dt, QKV_MODE)
    NSP = 2 if split3 else 1      # of x operand copies (hi, lo)
    ALU = mybir.AluOpType
    ACT = mybir.ActivationFunctionType
    from concourse.dve_ops import TENSOR_MASK as DVE_TENSOR_MASK

    nc = bass.Bass(num_devices=NCORES)
    ge_eng = getattr(nc, GE_ENG)

    # ---- I/O ----
    # x feature-major: [wi, split, kc, c(128), (t,s)]
    x_in = nc.dram_tensor("x_in", [NW, NSP, 2, 128, NTOK], qkv_dt, kind="ExternalInput")
    # w layouts [split, kc, c(128), f]
    wkv_in = nc.dram_tensor("wkv_in", [NSP, 2, 128, 512], qkv_dt, kind="ExternalInput")
    wq_in = nc.dram_tensor("wq_in", [NSP, 2, 128, 256], qkv_dt, kind="ExternalInput")
    # bias rows (pre-halved), bf16 hi+lo exact-ish splits
    bkv_b_in = nc.dram_tensor("bkv_b_in", [2, 512], bf16, kind="ExternalInput")
    bq_b_in = nc.dram_tensor("bq_b_in", [2, 256], bf16, kind="ExternalInput")
    wp_in = nc.dram_tensor("wp_in", [2, 128, 256], pj_dt, kind="ExternalInput")
    bp_in = nc.dram_tensor("bp_in", [2, 128, 1], f32, kind="ExternalInput")
    out_d = nc.dram_tensor("out_d", [NW, 2, 128, NTOK], f32, kind="ExternalOutput")

    with tile.TileContext(nc) as tc:
        with (
            tc.tile_pool(name="const", bufs=1) as constp,
            tc.tile_pool(name="xin", bufs=3) as xin_p,
            tc.tile_pool(name="hbuf", bufs=3) as h_p,
            tc.tile_pool(name="skv", bufs=2) as skv_p,
            tc.tile_pool(name="state", bufs=2) as st_p,
            tc.tile_pool(name="persist", bufs=1) as pers_p,
            tc.tile_pool(name="attn", bufs=2) as attn_p,
            tc.tile_pool(name="outs", bufs=2) as out_p,
            tc.tile_pool(name="dram", bufs=1, space="DRAM") as dram_p,
        ):
            # ---- constants / weights ----
            wkv_sb = constp.tile([128, NSP * 2 * 512], qkv_dt)
            wq_sb = constp.tile([128, NSP * 2 * 256], qkv_dt)
            for sp in range(NSP):
                for kc in range(2):
                    nc.sync.dma_start(
                        wkv_sb[:, (sp * 2 + kc) * 512:(sp * 2 + kc + 1) * 512],
                        wkv_in[sp, kc])
                    nc.sync.dma_start(
                        wq_sb[:, (sp * 2 + kc) * 256:(sp * 2 + kc + 1) * 256],
                        wq_in[sp, kc])
            wp_sb = constp.tile([128, 2 * 256], pj_dt)
            for kc in range(2):
                nc.sync.dma_start(wp_sb[:, kc * 256:(kc + 1) * 256], wp_in[kc])
            bp_sb = constp.tile([128, 2], f32)
            for ftc in range(2):
                nc.sync.dma_start(bp_sb[:, ftc:ftc + 1], bp_in[ftc])
            bkv_b = constp.tile([2, 512], bf16)     # rows: (hi, lo)
            bq_b = constp.tile([2, 256], bf16)
            nc.sync.dma_start(bkv_b[:], bkv_b_in[:])
            nc.sync.dma_start(bq_b[:], bq_b_in[:])
            ones128 = constp.tile([2, 128], bf16)
            nc.vector.memset(ones128[:], 1.0)
            ones512 = constp.tile([2, 512], bf16)
            nc.vector.memset(ones512[:], 1.0)

            # persistent across phases
            r_loc = pers_p.tile([128, 2048], f16)          # local R, (slab,t,wi,e)
            r_all = pers_p.tile([128, 8192], f16)          # gathered R, (slab,t,j,e)
            kv_w = [pers_p.tile([128, 256], f16, name=f"kvw{i}") for i in range(NW)]

            # ================= phase 1: per-window qkv + LIF + R =================
            ph1 = contextlib.ExitStack()
            ps_a = ph1.enter_context(tc.tile_pool(name="psA", bufs=2, space="PSUM"))
            ps_b = ph1.enter_context(tc.tile_pool(name="psB", bufs=1, space="PSUM"))
            ps_r = ph1.enter_context(tc.tile_pool(name="psr", bufs=1, space="PSUM"))
            for wi in range(NW):
                x_sb = xin_p.tile([128, NSP * 2 * NTOK], qkv_dt, tag="xsb")
                for sp in range(NSP):
                    for kc in range(2):
                        nc.sync.dma_start(
                            x_sb[:, (sp * 2 + kc) * NTOK:(sp * 2 + kc + 1) * NTOK],
                            x_in[wi, sp, kc])

                def xsl(sp, kc, lo, hi):
                    return x_sb[:, (sp * 2 + kc) * NTOK + lo:(sp * 2 + kc) * NTOK + hi]

                skv = skv_p.tile([128, 8 * 512], f16, tag="skv")

                # ---- pass A matmuls + LIF (k,v; token-major), both halves ----
                vkv = st_p.tile([128, 1024], f32, tag="vkv")
                lt = st_p.tile([128, 1024], f32, tag="ltkv")
                for t in range(T):
                    ps = ps_a.tile([128, 1024], f32, tag="psa")
                    for sh in range(2):
                        st = t * 2 + sh
                        reg = ps[:, sh * 512:(sh + 1) * 512]
                        first = True
                        if split3:
                            for kc in range(2):
                                for (sx, sw) in ((0, 0), (0, 1), (1, 0)):
                                    nc.tensor.matmul(
                                        reg,
                                        lhsT=xsl(sx, kc, st * 128, (st + 1) * 128),
                                        rhs=wkv_sb[:, (sw * 2 + kc) * 512:(sw * 2 + kc + 1) * 512],
                                        start=first, stop=False)
                                    first = False
                        else:
                            for kc in range(2):
                                nc.tensor.matmul(
                                    reg,
                                    lhsT=xsl(0, kc, st * 128, (st + 1) * 128),
                                    rhs=wkv_sb[:, kc * 512:(kc + 1) * 512],
                                    start=first, stop=False)
                                first = False
                        nc.tensor.matmul(reg, lhsT=ones128[:], rhs=bkv_b[:],
                                         start=False, stop=True)
                    # LIF step on [128,1024]; state vkv = 2*V ("W")
                    sksl = skv[:, (t * 2) * 512:(t * 2 + 2) * 512]
                    hsb = h_p.tile([128, 1024], f32, tag="hkv")
                    if t == 0:
                        nc.scalar.activation(hsb[:], ps[:], ACT.Copy, bias=0.0, scale=1.0)
                    else:
                        nc.vector.tensor_tensor(hsb[:], ps[:], vkv[:], op=ALU.add)
                    ge_eng.tensor_scalar(sksl, hsb[:], 1.0, None, ALU.is_ge)
                    if t < T - 1:
                        nc.vector.tensor_scalar(lt[:], hsb[:], 1.0, 0.5, ALU.is_lt, ALU.mult)
                        nc.vector.tensor_tensor(vkv[:], hsb[:], lt[:], op=ALU.mult)

                # ---- R = k^T v per (t, head): [d,e] blocks, col-tiled 4 heads ----
                for t in range(T):
                    for slab in range(2):
                        psr = ps_r.tile([128, 32], f32, tag="psr")
                        for hl in range(4):
                            h = slab * 4 + hl
                            for sh in range(2):
                                st = t * 2 + sh
                                nc.tensor.matmul(
                                    psr[32 * hl:32 * (hl + 1), :],
                                    lhsT=skv[:, st * 512 + h * 32: st * 512 + (h + 1) * 32],
                                    rhs=skv[:, st * 512 + 256 + h * 32: st * 512 + 256 + (h + 1) * 32],
                                    start=(sh == 0), stop=(sh == 1),
                                    tile_position=(0, 32 * hl),
                                )
                        nc.scalar.activation(
                            r_loc[:, ((slab * 4 + t) * 8 + wi) * 32:
                                  ((slab * 4 + t) * 8 + wi + 1) * 32],
                            psr[:], ACT.Copy, bias=0.0, scale=1.0)

            ph1.close()
            # ================= phase 2: exchange R, kv sums, attention, proj ======
            ph2 = contextlib.ExitStack()
            ps_at = ph2.enter_context(tc.tile_pool(name="psat", bufs=2, space="PSUM"))
            ps_pj = ph2.enter_context(tc.tile_pool(name="pspj", bufs=2, space="PSUM"))
            rb_in = dram_p.tile([128, 2048], f16)
            rb_out = dram_p.tile([4, 128, 2048], f16)
            nc.sync.dma_start(rb_in[:], r_loc[:])
            if sim_mode:
                for rk in range(4):
                    nc.sync.dma_start(rb_out[rk], rb_in[:])
            else:
                nc.gpsimd.collective_compute(
                    "AllGather",
                    mybir.AluOpType.bypass,
                    replica_groups=[[0, 1, 2, 3], [4, 5, 6, 7]],
                    ins=[rb_in[:].opt()],
                    outs=[rb_out[:].opt()],
                )
            # r_all free layout: (slab2, t4, j32, e32)
            r_all_v = r_all[:].rearrange("p (a t j e) -> p a t j e", a=2, t=4, j=32, e=32)
            for rk in range(4):
                src = rb_out[rk].rearrange("p (a t w e) -> p a t w e", a=2, t=4, w=8, e=32)
                nc.sync.dma_start(r_all_v[:, :, :, rk * 8:(rk + 1) * 8, :], src)

            # kv sums: routed gather baked per core, guarded by If on core id
            pid = None if sim_mode else nc.partition_id()
            for r in range(NCORES):
                if sim_mode and r != 0:
                    continue
                b_of = r // 4
                wg = r % 4
                with (contextlib.nullcontext() if sim_mode else tc.If(pid == r)):
                    for wl in range(NW):
                        wglob = wg * 8 + wl
                        idxs = [int(j) for j in routing_idx[b_of, wglob]]
                        eng = nc.gpsimd if (wl % 4 == 3) else nc.vector
                        dst = kv_w[wl][:]
                        src0 = r_all_v[:, :, :, idxs[0], :]
                        eng.tensor_copy(dst, src0)
                        for j in idxs[1:]:
                            eng.tensor_tensor(
                                dst, dst, r_all_v[:, :, :, j, :], op=ALU.add)

            # ============ segment 2: q matmuls + LIF-q + attention + proj ========
            ps_b2 = ph2.enter_context(tc.tile_pool(name="psB2", bufs=2, space="PSUM"))
            for wi in range(NW):
                x_sb = xin_p.tile([128, NSP * 2 * NTOK], qkv_dt, tag="xsb")
                for sp in range(NSP):
                    for kc in range(2):
                        nc.sync.dma_start(
                            x_sb[:, (sp * 2 + kc) * NTOK:(sp * 2 + kc + 1) * NTOK],
                            x_in[wi, sp, kc])

                def xsl(sp, kc, lo, hi):
                    return x_sb[:, (sp * 2 + kc) * NTOK + lo:(sp * 2 + kc) * NTOK + hi]

                sq_w = skv_p.tile([128, 2048], f16, tag="sqw")
                # ---- pass B matmuls + LIF (q; feature-major), both ftc batched ----
                vq = st_p.tile([128, 512], f32, tag="vq")
                ltq = st_p.tile([128, 512], f32, tag="ltq")
                for nch in range(2):
                    psq = ps_b2.tile([128, 1024], f32, tag="psb2")
                    for ftc in range(2):
                        reg = psq[:, ftc * 512:(ftc + 1) * 512]
                        first = True
                        if split3:
                            for kc in range(2):
                                for (sx, sw) in ((0, 0), (1, 0), (0, 1)):
                                    nc.tensor.matmul(
                                        reg,
                                        lhsT=wq_sb[:, (sw * 2 + kc) * 256 + ftc * 128:
                                                   (sw * 2 + kc) * 256 + (ftc + 1) * 128],
                                        rhs=xsl(sx, kc, nch * 512, (nch + 1) * 512),
                                        start=first, stop=False)
                                    first = False
                        else:
                            for kc in range(2):
                                nc.tensor.matmul(
                                    reg,
                                    lhsT=wq_sb[:, kc * 256 + ftc * 128:
                                               kc * 256 + (ftc + 1) * 128],
                                    rhs=xsl(0, kc, nch * 512, (nch + 1) * 512),
                                    start=first, stop=False)
                                first = False
                        nc.tensor.matmul(reg, lhsT=bq_b[:, ftc * 128:(ftc + 1) * 128],
                                         rhs=ones512[:], start=False, stop=True)
                    psq_v = psq[:].rearrange("p (f tl e) -> p f tl e", f=2, tl=2, e=256)
                    sq_v = sq_w[:].rearrange("p (f t e) -> p f t e", f=2, t=4, e=256)
                    vq_v = vq[:].rearrange("p (f e) -> p f e", f=2, e=256)
                    ltq_v = ltq[:].rearrange("p (f e) -> p f e", f=2, e=256)
                    for tl in range(2):
                        t = nch * 2 + tl
                        X = psq_v[:, :, tl, :]
                        sqsl = sq_v[:, :, t, :]
                        hq = h_p.tile([128, 512], f32, tag="hq")
                        hq_v = hq[:].rearrange("p (f e) -> p f e", f=2, e=256)
                        if t == 0:
                            nc.scalar.activation(hq_v, X, ACT.Copy, bias=0.0, scale=1.0)
                        else:
                            nc.vector.tensor_tensor(hq_v, X, vq_v, op=ALU.add)
                        ge_eng.tensor_scalar(sqsl, hq_v, 1.0, None, ALU.is_ge)
                        if t < T - 1:
                            nc.vector.tensor_scalar(ltq[:], hq[:], 1.0, 0.5, ALU.is_lt, ALU.mult)
                            nc.vector.tensor_tensor(vq[:], hq[:], ltq[:], op=ALU.mult)

                attn = attn_p.tile([128, 2 * NTOK], pj_dt, tag="attn")
                for t in range(T):
                    for slab in range(2):
                        psa = ps_at.tile([128, 256], f32, tag="psat")
                        for hl in range(4):
                            nc.tensor.matmul(
                                psa[32 * hl:32 * (hl + 1), :],
                                lhsT=kv_w[wi][32 * hl:32 * (hl + 1),
                                              (slab * 4 + t) * 32:
                                              (slab * 4 + t + 1) * 32],
                                rhs=sq_w[32 * hl:32 * (hl + 1),
                                         slab * NTOK + t * 256:
                                         slab * NTOK + (t + 1) * 256],
                                start=True, stop=True,
                                tile_position=(32 * hl, 32 * hl),
                            )
                        dst_at = attn[:, slab * NTOK + t * 256: slab * NTOK + (t + 1) * 256]
                        nc.scalar.activation(dst_at, psa[:], ACT.Copy, bias=0.0, scale=1.0)

                outsb = out_p.tile([128, 2 * NTOK], f32, tag="outsb")
                for cft in range(2):
                    for nch in range(2):
                        psp = ps_pj.tile([128, 512], f32, tag="pspj")
                        for kc in range(2):
                            nc.tensor.matmul(
                                psp[:],
                                lhsT=wp_sb[:, kc * 256 + cft * 128: kc * 256 + (cft + 1) * 128],
                                rhs=attn[:, kc * NTOK + nch * 512: kc * NTOK + (nch + 1) * 512],
                                start=(kc == 0), stop=(kc == 1),
                            )
                        nc.scalar.activation(
                            outsb[:, cft * NTOK + nch * 512: cft * NTOK + (nch + 1) * 512],
                            psp[:], ACT.Identity, bias=bp_sb[:, cft:cft + 1], scale=1.0)
                for cft in range(2):
                    nc.sync.dma_start(out_d[wi, cft], outsb[:, cft * NTOK:(cft + 1) * NTOK])
            ph2.close()

    _dedup_ldweights(nc, mybir)
    _split_sync_waits(nc, mybir, maxw=1)
    return nc


def _host_prepost(x, w_qkv, b_qkv):
    """Window partition, routing."""
    xw = x.reshape(T, B, WT, GT, WH, GH, WW, GW, C) \
          .transpose(0, 1, 2, 4, 6, 3, 5, 7, 8).reshape(T, B, W, S, C)
    xbar = xw.mean(axis=(0, 3))                      # [B, W, C]
    q_reg = xbar @ w_qkv[:, :C] + b_qkv[:C]
    k_reg = xbar @ w_qkv[:, C:2 * C] + b_qkv[C:2 * C]
    a_r = np.einsum('bwc,bvc->bwv', q_reg, k_reg)
    routing_idx = np.argsort(-a_r, axis=-1)[:, :, :TOPK]   # [B, W, TOPK]
    return xw, routing_idx


def _hi_lo16(a):
    hi = a.astype(ml_dtypes.float16 if hasattr(ml_dtypes, 'float16') else np.float16)
    hi = a.astype(np.float16)
    lo = (a - hi.astype(np.float32)).astype(np.float16)
    return hi, lo


def _hi_lo_bf(a):
    hi = a.astype(ml_dtypes.bfloat16)
    lo = (a - hi.astype(np.float32)).astype(ml_dtypes.bfloat16)
    return hi, lo


def kernel(x, w_qkv, b_qkv, w_proj, b_proj):
    x = np.ascontiguousarray(np.asarray(x, dtype=np.float32))
    w_qkv = np.asarray(w_qkv, dtype=np.float32)
    b_qkv = np.asarray(b_qkv, dtype=np.float32)
    w_proj = np.asarray(w_proj, dtype=np.float32)
    b_proj = np.asarray(b_proj, dtype=np.float32)

    xw, routing_idx = _host_prepost(x, w_qkv, b_qkv)

    key = (routing_idx.tobytes(), QKV_MODE, PJ_DT_NAME)
    if key not in _prog_cache:
        _prog_cache.clear()
        _prog_cache[key] = _build_program(routing_idx)
    nc = _prog_cache[key]

    split3 = QKV_MODE == "f16x3"
    np_qkv = np.float16 if split3 else np.float32

    # weights (shared across cores)
    wkv_half = (0.5 * w_qkv[:, C:]).astype(np.float32)
    wq_half = (0.5 * w_qkv[:, :C]).astype(np.float32)
    if split3:
        wkv_hi, wkv_lo = _hi_lo16(wkv_half)
        wkv_arr = np.stack([wkv_hi, wkv_lo]).reshape(2, 2, 128, 512)
        wq_hi, wq_lo = _hi_lo16(wq_half)
        wq_arr = np.stack([wq_hi, wq_lo]).reshape(2, 2, 128, 256)
    else:
        wkv_arr = wkv_half.reshape(1, 2, 128, 512)
        wq_arr = wq_half.reshape(1, 2, 128, 256)

    bkv_hi, bkv_lo = _hi_lo_bf((0.5 * b_qkv[C:]).astype(np.float32))
    bkv_arr = np.stack([bkv_hi, bkv_lo]).reshape(2, 512)
    bq_hi, bq_lo = _hi_lo_bf((0.5 * b_qkv[:C]).astype(np.float32))
    bq_arr = np.stack([bq_hi, bq_lo]).reshape(2, 256)

    wp = (SCALE * w_proj).reshape(2, 128, 256).astype(np.float32)
    bp = b_proj.reshape(2, 128, 1).astype(np.float32)

    in_maps = []
    for r in range(NCORES):
        b_of, wg = r // 4, r % 4
        xwc = xw[:, b_of, wg * 8:(wg + 1) * 8]              # [T, 8, S, C]
        xl32 = np.ascontiguousarray(
            xwc.transpose(1, 3, 0, 2).reshape(NW, 2, 128, NTOK))
        if split3:
            xhi = xl32.astype(np.float16)
            xlo = (xl32 - xhi.astype(np.float32)).astype(np.float16)
            xl = np.stack([xhi, xlo], axis=1)               # [NW, 2, 2, 128, NTOK]
        else:
            xl = xl32.reshape(NW, 1, 2, 128, NTOK)
        in_maps.append({
            "x_in": xl,
            "wkv_in": wkv_arr.astype(np_qkv), "wq_in": wq_arr.astype(np_qkv),
            "bkv_b_in": bkv_arr, "bq_b_in": bq_arr,
            "wp_in": wp, "bp_in": bp,
        })

    from concourse.bass_utils import run_bass_kernel_spmd
    res = run_bass_kernel_spmd(nc, in_maps, core_ids=list(range(NCORES)))

    # assemble output
    yw = np.empty((T, B, W, S, C), dtype=np.float32)
    for r in range(NCORES):
        b_of, wg = r // 4, r % 4
        o = res.results[r]["out_d"]                          # [NW, 2, 128, NTOK]
        o = o.reshape(NW, 2, 128, T, S).transpose(0, 3, 4, 1, 2).reshape(NW, T, S, C)
        for wl in range(NW):
            yw[:, b_of, wg * 8 + wl] = o[wl]

    y = yw.reshape(T, B, WT, WH, WW, GT, GH, GW, C) \
          .transpose(0, 1, 2, 5, 3, 6, 4, 7, 8).reshape(T, B, Lt, Lh, Lw, C)
    return y



# revision 31
# speedup vs baseline: 1.8621x; 1.8621x over previous
"""BiLevelRoutingAttention Trainium2 kernel (8-core SPMD).

Sharding: core r handles batch b = r//4 and windows w in [ (r%4)*8, (r%4)*8+8 ).
Routing (region top-k) is computed on host via linearity of the mean:
    q_region = mean_{t,s}(xw) @ Wq + bq  (exact up to fp reassociation).

Single-pass fp16 qkv matmuls (rel err budget 2e-2; this lands ~1e-3).
LIF is restructured around a bias-carrying state w = v + b:
    h_t = ps_t + 0.5*w_{t-1}   (ps = x@W/2 via pre-halved weights; the
                                0.5*w term is INJECTED into PSUM by the PE
                                with a 0.5*I stationary matmul)
    s_t = step(h_t)            (spike, read directly from PSUM)
    w_t = h_t*(h_t<1) + b      (fused scalar_tensor_tensor + f16 add)
Spikes are binary -> attention arithmetic is exact integer math in fp16.
Cross-window kv sums need R (per-region k^T v) from sibling cores ->
AllGather over each batch's 4-core group.

Attention uses a block-diagonal stationary (4 head-blocks of kv placed on
the PE diagonal via a DMA-materialized BD matrix) -> one 256-row matmul
per (slab,t) instead of four.

Spike engine per (t) is tunable: 'pool' (gpsimd is_ge, exact on HW),
'act' (sigmoid with huge scale; exact in saturation), 'dve' (is_ge fp16,
~2.6e-3 on HW). Env KSPK="aapp" style overrides (kv), KSPKQ (q).
"""

import numpy as np
import os as _os

# ---- problem constants (hardcoded per contract) ----
T, B, Lt, Lh, Lw, C = 4, 2, 8, 32, 32, 256
WT, WH, WW = 2, 4, 4
W = WT * WH * WW            # 32 windows
GT, GH, GW = Lt // WT, Lh // WH, Lw // WW
S = GT * GH * GW            # 256 tokens per window
H, D = 8, C // 8            # 8 heads, 32 dim
TOPK = 8
SCALE = float(D) ** -0.5
NCORES = 8
NW = 8                      # windows per core
NTOK = T * S                # 1024 token-instances per window

SPK_KV = _os.environ.get("KSPK", "pdad")    # per-t engine: a=act p=pool d=dve
SPK_Q = _os.environ.get("KSPKQ", "dddd")
ATTN_EVAC = _os.environ.get("KAEV", "dddd")  # act | dve | pool
SIGMA = 30000.0                              # sigmoid step sharpness

_prog_cache = {}


def _split_sync_waits(nc, mybir, maxw=1):
    """walrus in this container rejects >1 sync wait per instruction; split
    excess waits onto NoOp instructions inserted just before."""
    for bb in nc.main_func.blocks:
        new_list = []
        for ins in bb.instructions:
            si = ins.sync_info
            waits = list(si.on_wait) if si is not None and si.on_wait else []
            if len(waits) > maxw:
                extra = waits[:-maxw]
                keep = waits[-maxw:]
                idx = 0
                while extra:
                    chunk, extra = extra[:maxw], extra[maxw:]
                    nop = mybir.InstNoOp(name=f"{ins.name}-wsplit{idx}", ins=[], outs=[])
                    nop.engine = ins.engine
                    nop.sync_info = mybir.SyncInfo(on_wait=chunk, on_update=[])
                    new_list.append(nop)
                    idx += 1
                ins.sync_info = mybir.SyncInfo(
                    on_wait=keep,
                    on_update=list(si.on_update) if si.on_update else [],
                )
            new_list.append(ins)
        bb.instructions[:] = new_list


def _build_program(routing_idx, sim_mode=False):
    """routing_idx: [B, W, TOPK] int array (host-computed). Returns nc.
    sim_mode: no collective / no If-chain (single-core TimelineSim)."""
    import contextlib
    import concourse.bass as bass
    import concourse.mybir as mybir
    import concourse.tile as tile

    f32 = mybir.dt.float32
    f16 = mybir.dt.float16
    ALU = mybir.AluOpType
    ACT = mybir.ActivationFunctionType

    nc = bass.Bass(num_devices=NCORES)

    # ---- I/O ----
    # x feature-major: [wi, kc, c(128), (t,s)]
    x_in = nc.dram_tensor("x_in", [NW, 2, 128, NTOK], f16, kind="ExternalInput")
    wkv_in = nc.dram_tensor("wkv_in", [2, 128, 512], f16, kind="ExternalInput")
    wq_in = nc.dram_tensor("wq_in", [2, 128, 256], f16, kind="ExternalInput")
    bkvh_in = nc.dram_tensor("bkvh_in", [2, 1024], f16, kind="ExternalInput")
    bqh_in = nc.dram_tensor("bqh_in", [2, 256], f16, kind="ExternalInput")
    wp_in = nc.dram_tensor("wp_in", [2, 128, 256], f16, kind="ExternalInput")
    bp_in = nc.dram_tensor("bp_in", [2, 128, 1], f32, kind="ExternalInput")
    out_d = nc.dram_tensor("out_d", [NW, 2, 128, NTOK], f32, kind="ExternalOutput")

    with tile.TileContext(nc) as tc:
        with (
            tc.tile_pool(name="const", bufs=1) as constp,
            tc.tile_pool(name="xin", bufs=8) as xin_p,
            tc.tile_pool(name="skv", bufs=2) as skv_p,
            tc.tile_pool(name="state", bufs=3) as st_p,
            tc.tile_pool(name="persist", bufs=1) as pers_p,
            tc.tile_pool(name="outs", bufs=2) as out_p,
            tc.tile_pool(name="gtmp", bufs=2) as gt_p,
            tc.tile_pool(name="dram", bufs=1, space="DRAM") as dram_p,
        ):
            # ---- constants / weights (kv-critical ones first; rest are
            # DMA'd after the first x tiles to keep the PE fed early) ----
            wkv_sb = constp.tile([128, 2 * 512], f16)
            for kc in range(2):
                nc.sync.dma_start(wkv_sb[:, kc * 512:(kc + 1) * 512], wkv_in[kc])
            bkvh_c = constp.tile([2, 1024], f16)     # b_kv/4 rows x2, (sh,feat)
            nc.sync.dma_start(bkvh_c[:], bkvh_in[:])
            wq_sb = constp.tile([128, 2 * 256], f16)
            wp_sb = constp.tile([128, 2 * 256], f16)
            bp_sb = constp.tile([128, 2], f32)
            bqh_c = constp.tile([2, 256], f16)       # b_q/4 rows x2 (feature idx)
            ones1 = constp.tile([2, 256], f16)
            nc.vector.memset(ones1[:], 1.0)
            nsig = constp.tile([128, 1], f32)
            nc.vector.memset(nsig[:], -SIGMA)

            def load_late_consts():
                for kc in range(2):
                    nc.sync.dma_start(wq_sb[:, kc * 256:(kc + 1) * 256], wq_in[kc])
                    nc.sync.dma_start(wp_sb[:, kc * 256:(kc + 1) * 256], wp_in[kc])
                for ftc in range(2):
                    nc.sync.dma_start(bp_sb[:, ftc:ftc + 1], bp_in[ftc])
                nc.sync.dma_start(bqh_c[:], bqh_in[:])

            # persistent across phases
            r_loc = pers_p.tile([128, 2048], f16)          # local R, (slab,t,wi,e)
            r_all = pers_p.tile([128, 8192], f16)          # gathered R, (slab,t,j,e)
            kv_w_all = pers_p.tile([128, NW * 256], f16)
            sq_w = [pers_p.tile([128, 2048], f16, name=f"sqw{i}") for i in range(NW)]
            # block-diagonal kv for all windows; off-diagonal zeros are set
            # once and persist (diag blocks land at fixed offsets).
            bd_all = pers_p.tile([128, NW * 1024], f16)
            nc.vector.memset(bd_all[:], 0.0)

            def spike(eng, dst, src):
                if eng == 'a':
                    nc.scalar.activation(dst, src, ACT.Sigmoid,
                                         bias=nsig[:, 0:1], scale=SIGMA)
                elif eng == 'p':
                    nc.gpsimd.tensor_scalar(dst, src, 1.0, None, ALU.is_ge)
                else:
                    nc.vector.tensor_scalar(dst, src, 1.0, None, ALU.is_ge)

            rb_inA = dram_p.tile([128, 1024], f16)
            rb_outA = dram_p.tile([4, 128, 1024], f16)
            rb_inB = dram_p.tile([128, 1024], f16)
            rb_outB = dram_p.tile([4, 128, 1024], f16)

            def exchange_half(half):
                rb_i, rb_o = (rb_inA, rb_outA) if half == 0 else (rb_inB, rb_outB)
                r_loc_v = r_loc[:].rearrange("p (a w e) -> p a w e", a=8, w=8, e=32)
                nc.sync.dma_start(
                    rb_i[:].rearrange("p (a w e) -> p a w e", a=8, w=4, e=32),
                    r_loc_v[:, :, half * 4:(half + 1) * 4, :])
                if sim_mode:
                    for rk in range(4):
                        nc.sync.dma_start(rb_o[rk], rb_i[:])
                else:
                    nc.gpsimd.collective_compute(
                        "AllGather",
                        mybir.AluOpType.bypass,
                        replica_groups=[[0, 1, 2, 3], [4, 5, 6, 7]],
                        ins=[rb_i[:].opt()],
                        outs=[rb_o[:].opt()],
                    )
                r_all_vv = r_all[:].rearrange("p (a t j e) -> p a t j e", a=2, t=4, j=32, e=32)
                for rk in range(4):
                    srcv = rb_o[rk].rearrange("p (a w e) -> p a w e", a=8, w=4, e=32)                         .rearrange("p (s t) w e -> p s t w e", s=2, t=4)
                    nc.sync.dma_start(
                        r_all_vv[:, :, :, rk * 8 + half * 4: rk * 8 + (half + 1) * 4, :],
                        srcv)

            # ================= phase 1: qkv + LIF + R =================
            ph1 = contextlib.ExitStack()
            ps_a = ph1.enter_context(tc.tile_pool(name="psA", bufs=2, space="PSUM"))
            ps_q = ph1.enter_context(tc.tile_pool(name="psQ", bufs=2, space="PSUM"))
            ps_r = ph1.enter_context(tc.tile_pool(name="psr", bufs=2, space="PSUM"))
            x_tiles = []
            for wi in range(NW):
                x_sb = xin_p.tile([128, 2048], f16, tag="xsb")
                x_tiles.append(x_sb)
                for kc in range(2):
                    nc.sync.dma_start(x_sb[:, kc * 1024:(kc + 1) * 1024], x_in[wi, kc])

                if wi == 1:
                    load_late_consts()
                skv = skv_p.tile([128, 4096], f16, tag="skv")
                v16h = st_p.tile([128, 1024], f16, tag="vkv")   # kv v/2 state
                for t in range(T):
                    # ---- kv matmuls (token-major); bias via ones-row mm ----
                    ps = ps_a.tile([128, 1024], f32, tag="psa")
                    for sh in range(2):
                        reg = ps[:, sh * 512:(sh + 1) * 512]
                        for kc in range(2):
                            nc.tensor.matmul(
                                reg,
                                lhsT=x_sb[:, kc * 1024 + t * 256 + sh * 128:
                                          kc * 1024 + t * 256 + (sh + 1) * 128],
                                rhs=wkv_sb[:, kc * 512:(kc + 1) * 512],
                                start=(kc == 0), stop=False)
                        nc.tensor.matmul(
                            reg, lhsT=ones1[:, :128],
                            rhs=bkvh_c[:, sh * 512:(sh + 1) * 512],
                            start=False, stop=True)
                    # evacuate h-partial = x@W/2 + b/2 (Act); h = hps + v/2
                    hps = st_p.tile([128, 1024], f16, tag="hps")
                    nc.scalar.activation(hps[:], ps[:], ACT.Copy, bias=0.0, scale=1.0)
                    if t == 0:
                        hkv = hps
                    else:
                        hkv = st_p.tile([128, 1024], f16, tag="hkv")
                        nc.vector.tensor_tensor(hkv[:], hps[:], v16h[:], op=ALU.add)
                    s_sl = skv[:, t * 1024:(t + 1) * 1024]
                    spike(SPK_KV[t], s_sl, hkv[:])
                    if t < T - 1:
                        lt = st_p.tile([128, 1024], f16, tag="lt")
                        nc.vector.tensor_scalar(lt[:], hkv[:], 1.0, 0.5, ALU.is_lt, ALU.mult)
                        nc.vector.tensor_tensor(v16h[:], hkv[:], lt[:], op=ALU.mult)

                # ---- R = k^T v per (t, head): col-tiled 4 heads ----
                for slab in range(2):
                    psr = ps_r.tile([128, 128], f32, tag="psr")
                    for t in range(T):
                        for hl in range(4):
                            h = slab * 4 + hl
                            for sh in range(2):
                                base = t * 1024 + sh * 512
                                nc.tensor.matmul(
                                    psr[32 * hl:32 * (hl + 1), t * 32:(t + 1) * 32],
                                    lhsT=skv[:, base + h * 32: base + (h + 1) * 32],
                                    rhs=skv[:, base + 256 + h * 32: base + 256 + (h + 1) * 32],
                                    start=(sh == 0), stop=(sh == 1),
                                    tile_position=(0, 32 * hl),
                                )
                    dst = r_loc[:].rearrange("p (a w e) -> p a w e", a=8, w=8, e=32)[
                        :, slab * 4:(slab + 1) * 4, wi, :]
                    nc.vector.tensor_copy(dst, psr[:])
                if wi == 3:
                    exchange_half(0)
                elif wi == 7:
                    exchange_half(1)

            # r_all free layout: (slab2, t4, j32, e32)
            r_all_v = r_all[:].rearrange("p (a t j e) -> p a t j e", a=2, t=4, j=32, e=32)

            # ---- q-pass (feature-major) interleaved with gather emission ----
            def emit_q_window(wi):
                x_sb = x_tiles[wi]
                sqw = sq_w[wi]
                sqw_v = sqw[:].rearrange("p (f t e) -> p f t e", f=2, t=4, e=256)
                vq16h = st_p.tile([128, 512], f16, tag="vq")    # q v/2 state
                for t in range(T):
                    psq = ps_q.tile([128, 512], f32, tag="psq")
                    for fc in range(2):
                        reg = psq[:, fc * 256:(fc + 1) * 256]
                        for kc in range(2):
                            nc.tensor.matmul(
                                reg,
                                lhsT=wq_sb[:, kc * 256 + fc * 128:
                                           kc * 256 + (fc + 1) * 128],
                                rhs=x_sb[:, kc * 1024 + t * 256:
                                         kc * 1024 + (t + 1) * 256],
                                start=(kc == 0), stop=False)
                        nc.tensor.matmul(
                            reg, lhsT=bqh_c[:, fc * 128:(fc + 1) * 128],
                            rhs=ones1[:], start=False, stop=True)
                    hqps = st_p.tile([128, 512], f16, tag="hqps")
                    nc.scalar.activation(hqps[:], psq[:], ACT.Copy, bias=0.0, scale=1.0)
                    if t == 0:
                        hq = hqps
                    else:
                        hq = st_p.tile([128, 512], f16, tag="hq")
                        nc.vector.tensor_tensor(hq[:], hqps[:], vq16h[:], op=ALU.add)
                    sq_sl = sqw_v[:, :, t, :]
                    spike(SPK_Q[t], sq_sl, hq[:])
                    if t < T - 1:
                        ltq = st_p.tile([128, 512], f16, tag="ltq")
                        nc.vector.tensor_scalar(ltq[:], hq[:], 1.0, 0.5, ALU.is_lt, ALU.mult)
                        nc.vector.tensor_tensor(vq16h[:], hq[:], ltq[:], op=ALU.mult)

            # kv sums: routed gather baked per core, guarded by If on core id
            pid = None if sim_mode else nc.partition_id()

            def emit_gather(gather_windows):
                for r in range(NCORES):
                    if sim_mode and r != 0:
                        continue
                    b_of = r // 4
                    wg = r % 4
                    with (contextlib.nullcontext() if sim_mode else tc.If(pid == r)):
                        for wl in gather_windows:
                            wglob = wg * 8 + wl
                            idxs = [int(j) for j in routing_idx[b_of, wglob]]
                            dst = kv_w_all[:, wl * 256:(wl + 1) * 256]
                            # tree reduction, levels split across DVE and Pool
                            g1a = gt_p.tile([128, 256], f16, tag="g1a")
                            g1b = gt_p.tile([128, 256], f16, tag="g1b")
                            nc.vector.tensor_tensor(
                                g1a[:], r_all_v[:, :, :, idxs[0], :],
                                r_all_v[:, :, :, idxs[1], :], op=ALU.add)
                            nc.vector.tensor_tensor(
                                g1a[:], g1a[:],
                                r_all_v[:, :, :, idxs[2], :], op=ALU.add)
                            nc.vector.tensor_tensor(
                                g1a[:], g1a[:],
                                r_all_v[:, :, :, idxs[3], :], op=ALU.add)
                            g1c = gt_p.tile([128, 256], f16, tag="g1c")
                            nc.gpsimd.tensor_tensor(
                                g1b[:], r_all_v[:, :, :, idxs[4], :],
                                r_all_v[:, :, :, idxs[5], :], op=ALU.add)
                            nc.gpsimd.tensor_tensor(
                                g1c[:], r_all_v[:, :, :, idxs[6], :],
                                r_all_v[:, :, :, idxs[7], :], op=ALU.add)
                            nc.vector.tensor_tensor(g1a[:], g1a[:], g1b[:], op=ALU.add)
                            nc.vector.tensor_tensor(dst, g1a[:], g1c[:], op=ALU.add)

            for wi in range(NW):
                emit_q_window(wi)
                if wi % 2 == 1:
                    emit_gather([wi - 1, wi])
            ph1.close()
            ph2 = contextlib.ExitStack()
            ps_at = ph2.enter_context(tc.tile_pool(name="psat", bufs=3, space="PSUM"))
            ps_pj = ph2.enter_context(tc.tile_pool(name="pspj", bufs=3, space="PSUM"))

            # batched block-diagonal materialization: 2 batches x 4 DMAs
            kvw_v = kv_w_all[:].rearrange("p (w st e) -> p w st e", w=NW, st=8, e=32)
            bd_v = bd_all[:].rearrange("p (w st c) -> p w st c", w=NW, st=8, c=128)
            for hb in range(2):
                for hl in range(4):
                    nc.sync.dma_start(
                        bd_v[32 * hl:32 * (hl + 1), hb * 4:(hb + 1) * 4, :,
                             32 * hl:32 * (hl + 1)],
                        kvw_v[32 * hl:32 * (hl + 1), hb * 4:(hb + 1) * 4])

            # ============ segment 2: attention sweep, then proj sweep ========
            attn_tiles = []
            for wi in range(NW):
                bd = bd_all[:, wi * 1024:(wi + 1) * 1024]
                sqw = sq_w[wi]
                attn = xin_p.tile([128, 2 * NTOK], f16, tag="xsb")
                attn_tiles.append(attn)
                for th in range(2):
                    for slab in range(2):
                        psa = ps_at.tile([128, 512], f32, tag="psat")
                        for tl in range(2):
                            t = th * 2 + tl
                            st = slab * 4 + t
                            nc.tensor.matmul(
                                psa[:, tl * 256:(tl + 1) * 256],
                                lhsT=bd[:, st * 128:(st + 1) * 128],
                                rhs=sqw[:, slab * NTOK + t * 256:
                                        slab * NTOK + (t + 1) * 256],
                                start=True, stop=True,
                            )
                        dst_at = attn[:, slab * NTOK + th * 512:
                                      slab * NTOK + (th + 1) * 512]
                        ev = ATTN_EVAC[(wi * 4 + th * 2 + slab) % len(ATTN_EVAC)]
                        if ev == "a":
                            nc.scalar.activation(dst_at, psa[:], ACT.Copy,
                                                 bias=0.0, scale=1.0)
                        elif ev == "p":
                            nc.gpsimd.tensor_copy(dst_at, psa[:])
                        else:
                            nc.vector.tensor_copy(dst_at, psa[:])

            for wi in range(NW):
                attn = attn_tiles[wi]
                outsb = out_p.tile([128, 2 * NTOK], f32, tag="outsb")
                for cft in range(2):
                    for nch in range(2):
                        psp = ps_pj.tile([128, 512], f32, tag="pspj")
                        for kc in range(2):
                            nc.tensor.matmul(
                                psp[:],
                                lhsT=wp_sb[:, kc * 256 + cft * 128: kc * 256 + (cft + 1) * 128],
                                rhs=attn[:, kc * NTOK + nch * 512: kc * NTOK + (nch + 1) * 512],
                                start=(kc == 0), stop=(kc == 1),
                            )
                        nc.scalar.activation(
                            outsb[:, cft * NTOK + nch * 512: cft * NTOK + (nch + 1) * 512],
                            psp[:], ACT.Identity, bias=bp_sb[:, cft:cft + 1], scale=1.0)
                outd_v = out_d[wi].rearrange("c p (n f) -> c p n f", n=2, f=512)
                for cft in range(2):
                    for nch in range(2):
                        nc.sync.dma_start(
                            outd_v[cft, :, nch, :],
                            outsb[:, cft * NTOK + nch * 512: cft * NTOK + (nch + 1) * 512])
            ph2.close()

    _split_sync_waits(nc, mybir, maxw=1)
    return nc


def _host_prepost(x, w_qkv, b_qkv):
    """Window partition, routing."""
    xw = x.reshape(T, B, WT, GT, WH, GH, WW, GW, C) \
          .transpose(0, 1, 2, 4, 6, 3, 5, 7, 8).reshape(T, B, W, S, C)
    xbar = xw.mean(axis=(0, 3))                      # [B, W, C]
    q_reg = xbar @ w_qkv[:, :C] + b_qkv[:C]
    k_reg = xbar @ w_qkv[:, C:2 * C] + b_qkv[C:2 * C]
    a_r = np.einsum('bwc,bvc->bwv', q_reg, k_reg)
    routing_idx = np.argsort(-a_r, axis=-1)[:, :, :TOPK]   # [B, W, TOPK]
    return xw, routing_idx


def kernel(x, w_qkv, b_qkv, w_proj, b_proj):
    x = np.ascontiguousarray(np.asarray(x, dtype=np.float32))
    w_qkv = np.asarray(w_qkv, dtype=np.float32)
    b_qkv = np.asarray(b_qkv, dtype=np.float32)
    w_proj = np.asarray(w_proj, dtype=np.float32)
    b_proj = np.asarray(b_proj, dtype=np.float32)

    xw, routing_idx = _host_prepost(x, w_qkv, b_qkv)

    key = (routing_idx.tobytes(), SPK_KV, SPK_Q, ATTN_EVAC)
    if key not in _prog_cache:
        _prog_cache.clear()
        _prog_cache[key] = _build_program(routing_idx)
    nc = _prog_cache[key]

    # weights (shared across cores), pre-halved for the LIF /TAU
    wkv_arr = (0.5 * w_qkv[:, C:]).reshape(2, 128, 512).astype(np.float16)
    wq_arr = (0.5 * w_qkv[:, :C]).reshape(2, 128, 256).astype(np.float16)
    bkvh_row = np.tile((0.25 * b_qkv[C:]).astype(np.float16), 2).reshape(1, 1024)
    bkvh_arr = np.concatenate([bkvh_row, bkvh_row], axis=0)          # [2,1024]
    bqh_row = (0.25 * b_qkv[:C]).astype(np.float16).reshape(1, 256)
    bqh_arr = np.concatenate([bqh_row, bqh_row], axis=0)             # [2,256]
    wp = (SCALE * w_proj).reshape(2, 128, 256).astype(np.float16)
    bp = b_proj.reshape(2, 128, 1).astype(np.float32)

    in_maps = []
    for r in range(NCORES):
        b_of, wg = r // 4, r % 4
        xwc = xw[:, b_of, wg * 8:(wg + 1) * 8]              # [T, 8, S, C]
        xl = np.ascontiguousarray(
            xwc.transpose(1, 3, 0, 2).reshape(NW, 2, 128, NTOK)).astype(np.float16)
        in_maps.append({
            "x_in": xl,
            "wkv_in": wkv_arr, "wq_in": wq_arr,
            "bkvh_in": bkvh_arr, "bqh_in": bqh_arr,
            "wp_in": wp, "bp_in": bp,
        })

    from concourse.bass_utils import run_bass_kernel_spmd
    res = run_bass_kernel_spmd(nc, in_maps, core_ids=list(range(NCORES)))

    # assemble output
    yw = np.empty((T, B, W, S, C), dtype=np.float32)
    for r in range(NCORES):
        b_of, wg = r // 4, r % 4
        o = res.results[r]["out_d"]                          # [NW, 2, 128, NTOK]
        o = o.reshape(NW, 2, 128, T, S).transpose(0, 3, 4, 1, 2).reshape(NW, T, S, C)
        for wl in range(NW):
            yw[:, b_of, wg * 8 + wl] = o[wl]

    y = yw.reshape(T, B, WT, WH, WW, GT, GH, GW, C) \
          .transpose(0, 1, 2, 5, 3, 6, 4, 7, 8).reshape(T, B, Lt, Lh, Lw, C)
    return y


# revision 39
# speedup vs baseline: 1.9265x; 1.0346x over previous
"""BiLevelRoutingAttention Trainium2 kernel (8-core SPMD).

Sharding: core r handles batch b = r//4 and windows w in [ (r%4)*8, (r%4)*8+8 ).
Routing (region top-k) is computed on host via linearity of the mean:
    q_region = mean_{t,s}(xw) @ Wq + bq  (exact up to fp reassociation).

Single-pass fp16 qkv matmuls (rel-err budget 2e-2; this lands ~3e-3).
LIF per timestep (TAU=2, pre-halved weights so ps = x@W/2 + b/2):
    hps = Act.Copy(ps)            (PSUM evacuation to fp16 SBUF)
    h   = hps + v/2               (DVE fp16 TT, v/2 kept as state)
    s   = step(h)                 (engine per t: Pool is_ge exact /
                                   Act sigmoid(sigma*(h-1)) / DVE is_ge fp16)
    v/2 = h*(h<1)*0.5             (DVE TS + TT, fp16 fast modes)
Biases enter via tiny ones-row matmuls accumulated into PSUM (K=2).
Spikes are binary -> attention arithmetic is exact integer math in fp16.

Structure: [kv pass + R for all 8 windows, with the R AllGather split in
two halves overlapped into phase 1] -> [q pass overlapping the exchange,
with per-window routed-gather trees (DVE+Pool) interleaved] -> [batched
block-diagonal kv materialization via 8 DMAs] -> [attention sweep: one
256-row matmul per (slab,t) against the BD stationary] -> [proj sweep].
Engine assignment knobs: KSPK/KSPKQ (spike engine per t: a/p/d), KAEV
(attn evacuation engines).
"""
import numpy as np
import os as _os

# ---- problem constants (hardcoded per contract) ----
T, B, Lt, Lh, Lw, C = 4, 2, 8, 32, 32, 256
WT, WH, WW = 2, 4, 4
W = WT * WH * WW            # 32 windows
GT, GH, GW = Lt // WT, Lh // WH, Lw // WW
S = GT * GH * GW            # 256 tokens per window
H, D = 8, C // 8            # 8 heads, 32 dim
TOPK = 8
SCALE = float(D) ** -0.5
NCORES = 8
NW = 8                      # windows per core
NTOK = T * S                # 1024 token-instances per window

SPK_KV = _os.environ.get("KSPK", "pddd")    # per-t engine: a=act p=pool d=dve
SPK_Q = _os.environ.get("KSPKQ", "appp")
ATTN_EVAC = _os.environ.get("KAEV", "dddd")  # act | dve | pool
SIGMA = 30000.0                              # sigmoid step sharpness

_prog_cache = {}


def _split_sync_waits(nc, mybir, maxw=1):
    """walrus in this container rejects >1 sync wait per instruction; split
    excess waits onto NoOp instructions inserted just before."""
    for bb in nc.main_func.blocks:
        new_list = []
        for ins in bb.instructions:
            si = ins.sync_info
            waits = list(si.on_wait) if si is not None and si.on_wait else []
            if len(waits) > maxw:
                extra = waits[:-maxw]
                keep = waits[-maxw:]
                idx = 0
                while extra:
                    chunk, extra = extra[:maxw], extra[maxw:]
                    nop = mybir.InstNoOp(name=f"{ins.name}-wsplit{idx}", ins=[], outs=[])
                    nop.engine = ins.engine
                    nop.sync_info = mybir.SyncInfo(on_wait=chunk, on_update=[])
                    new_list.append(nop)
                    idx += 1
                ins.sync_info = mybir.SyncInfo(
                    on_wait=keep,
                    on_update=list(si.on_update) if si.on_update else [],
                )
            new_list.append(ins)
        bb.instructions[:] = new_list


def _build_program(routing_idx, sim_mode=False):
    """routing_idx: [B, W, TOPK] int array (host-computed). Returns nc.
    sim_mode: no collective / no If-chain (single-core TimelineSim)."""
    import contextlib
    import concourse.bass as bass
    import concourse.mybir as mybir
    import concourse.tile as tile

    f32 = mybir.dt.float32
    f16 = mybir.dt.float16
    ALU = mybir.AluOpType
    ACT = mybir.ActivationFunctionType

    nc = bass.Bass(num_devices=NCORES)

    # ---- I/O ----
    # x feature-major: [wi, kc, c(128), (t,s)]
    x_in = nc.dram_tensor("x_in", [NW, 2, 128, NTOK], f16, kind="ExternalInput")
    wkv_in = nc.dram_tensor("wkv_in", [2, 128, 512], f16, kind="ExternalInput")
    wq_in = nc.dram_tensor("wq_in", [2, 128, 256], f16, kind="ExternalInput")
    bkvh_in = nc.dram_tensor("bkvh_in", [2, 1024], f16, kind="ExternalInput")
    bqh_in = nc.dram_tensor("bqh_in", [2, 256], f16, kind="ExternalInput")
    wp_in = nc.dram_tensor("wp_in", [2, 128, 256], f16, kind="ExternalInput")
    bp_in = nc.dram_tensor("bp_in", [2, 128, 1], f32, kind="ExternalInput")
    out_d = nc.dram_tensor("out_d", [NW, 2, 128, NTOK], f32, kind="ExternalOutput")

    with tile.TileContext(nc) as tc:
        with (
            tc.tile_pool(name="const", bufs=1) as constp,
            tc.tile_pool(name="xin", bufs=8) as xin_p,
            tc.tile_pool(name="skv", bufs=2) as skv_p,
            tc.tile_pool(name="state", bufs=3) as st_p,
            tc.tile_pool(name="persist", bufs=1) as pers_p,
            tc.tile_pool(name="outs", bufs=2) as out_p,
            tc.tile_pool(name="gtmp", bufs=2) as gt_p,
            tc.tile_pool(name="dram", bufs=1, space="DRAM") as dram_p,
        ):
            # ---- constants / weights (kv-critical ones first; rest are
            # DMA'd after the first x tiles to keep the PE fed early) ----
            wkv_sb = constp.tile([128, 2 * 512], f16)
            for kc in range(2):
                nc.sync.dma_start(wkv_sb[:, kc * 512:(kc + 1) * 512], wkv_in[kc])
            bkvh_c = constp.tile([2, 1024], f16)     # b_kv/4 rows x2, (sh,feat)
            nc.sync.dma_start(bkvh_c[:], bkvh_in[:])
            wq_sb = constp.tile([128, 2 * 256], f16)
            wp_sb = constp.tile([128, 2 * 256], f16)
            bp_sb = constp.tile([128, 2], f32)
            bqh_c = constp.tile([2, 256], f16)       # b_q/4 rows x2 (feature idx)
            ones1 = constp.tile([2, 256], f16)
            nc.vector.memset(ones1[:], 1.0)
            nsig = constp.tile([128, 1], f32)
            nc.vector.memset(nsig[:], -SIGMA)

            def load_late_consts():
                for kc in range(2):
                    nc.sync.dma_start(wq_sb[:, kc * 256:(kc + 1) * 256], wq_in[kc])
                    nc.sync.dma_start(wp_sb[:, kc * 256:(kc + 1) * 256], wp_in[kc])
                for ftc in range(2):
                    nc.sync.dma_start(bp_sb[:, ftc:ftc + 1], bp_in[ftc])
                nc.sync.dma_start(bqh_c[:], bqh_in[:])

            # persistent across phases
            r_loc = pers_p.tile([128, 2048], f16)          # local R, (slab,t,wi,e)
            r_all = pers_p.tile([128, 8192], f16)          # gathered R, (slab,t,j,e)
            kv_w_all = pers_p.tile([128, NW * 256], f16)
            sq_w = [pers_p.tile([128, 2048], f16, name=f"sqw{i}") for i in range(NW)]
            # block-diagonal kv for all windows; off-diagonal zeros are set
            # once and persist (diag blocks land at fixed offsets).
            bd_all = pers_p.tile([128, NW * 1024], f16)
            nc.gpsimd.memset(bd_all[:], 0.0)

            def spike(eng, dst, src):
                if eng == 'a':
                    nc.scalar.activation(dst, src, ACT.Sigmoid,
                                         bias=nsig[:, 0:1], scale=SIGMA)
                elif eng == 'p':
                    nc.gpsimd.tensor_scalar(dst, src, 1.0, None, ALU.is_ge)
                else:
                    nc.vector.tensor_scalar(dst, src, 1.0, None, ALU.is_ge)

            rb_inA = dram_p.tile([128, 1024], f16)
            rb_outA = dram_p.tile([4, 128, 1024], f16)
            rb_inB = dram_p.tile([128, 1024], f16)
            rb_outB = dram_p.tile([4, 128, 1024], f16)

            def exchange_half(half):
                rb_i, rb_o = (rb_inA, rb_outA) if half == 0 else (rb_inB, rb_outB)
                r_loc_v = r_loc[:].rearrange("p (a w e) -> p a w e", a=8, w=8, e=32)
                nc.sync.dma_start(
                    rb_i[:].rearrange("p (a w e) -> p a w e", a=8, w=4, e=32),
                    r_loc_v[:, :, half * 4:(half + 1) * 4, :])
                if sim_mode:
                    for rk in range(4):
                        nc.sync.dma_start(rb_o[rk], rb_i[:])
                else:
                    nc.gpsimd.collective_compute(
                        "AllGather",
                        mybir.AluOpType.bypass,
                        replica_groups=[[0, 1, 2, 3], [4, 5, 6, 7]],
                        ins=[rb_i[:].opt()],
                        outs=[rb_o[:].opt()],
                    )
                r_all_vv = r_all[:].rearrange("p (a t j e) -> p a t j e", a=2, t=4, j=32, e=32)
                for rk in range(4):
                    srcv = rb_o[rk].rearrange("p (a w e) -> p a w e", a=8, w=4, e=32)                         .rearrange("p (s t) w e -> p s t w e", s=2, t=4)
                    nc.sync.dma_start(
                        r_all_vv[:, :, :, rk * 8 + half * 4: rk * 8 + (half + 1) * 4, :],
                        srcv)

            # ================= phase 1: qkv + LIF + R =================
            ph1 = contextlib.ExitStack()
            ps_a = ph1.enter_context(tc.tile_pool(name="psA", bufs=2, space="PSUM"))
            ps_q = ph1.enter_context(tc.tile_pool(name="psQ", bufs=2, space="PSUM"))
            ps_r = ph1.enter_context(tc.tile_pool(name="psr", bufs=2, space="PSUM"))
            x_tiles = []
            for wi in range(NW):
                x_sb = xin_p.tile([128, 2048], f16, tag="xsb")
                x_tiles.append(x_sb)
                for half in range(2):
                    for kc in range(2):
                        nc.sync.dma_start(
                            x_sb[:, kc * 1024 + half * 512: kc * 1024 + (half + 1) * 512],
                            x_in[wi, kc, :, half * 512:(half + 1) * 512])

                if wi == 1:
                    load_late_consts()
                skv = skv_p.tile([128, 4096], f16, tag="skv")
                v16h = st_p.tile([128, 1024], f16, tag="vkv")   # kv v/2 state
                for t in range(T):
                    # ---- kv matmuls (token-major); bias via ones-row mm ----
                    ps = ps_a.tile([128, 1024], f32, tag="psa")
                    for sh in range(2):
                        reg = ps[:, sh * 512:(sh + 1) * 512]
                        for kc in range(2):
                            nc.tensor.matmul(
                                reg,
                                lhsT=x_sb[:, kc * 1024 + t * 256 + sh * 128:
                                          kc * 1024 + t * 256 + (sh + 1) * 128],
                                rhs=wkv_sb[:, kc * 512:(kc + 1) * 512],
                                start=(kc == 0), stop=False)
                        nc.tensor.matmul(
                            reg, lhsT=ones1[:, :128],
                            rhs=bkvh_c[:, sh * 512:(sh + 1) * 512],
                            start=False, stop=True)
                    # evacuate h-partial = x@W/2 + b/2 (Act); h = hps + v/2
                    hps = st_p.tile([128, 1024], f16, tag="hps")
                    nc.scalar.activation(hps[:], ps[:], ACT.Copy, bias=0.0, scale=1.0)
                    if t == 0:
                        hkv = hps
                    else:
                        hkv = st_p.tile([128, 1024], f16, tag="hkv")
                        nc.vector.tensor_tensor(hkv[:], hps[:], v16h[:], op=ALU.add)
                    s_sl = skv[:, t * 1024:(t + 1) * 1024]
                    spike(SPK_KV[t], s_sl, hkv[:])
                    if t < T - 1:
                        lt = st_p.tile([128, 1024], f16, tag="lt")
                        nc.vector.tensor_scalar(lt[:], hkv[:], 1.0, 0.5, ALU.is_lt, ALU.mult)
                        nc.vector.tensor_tensor(v16h[:], hkv[:], lt[:], op=ALU.mult)

                # ---- R = k^T v per (t, head): col-tiled 4 heads ----
                for slab in range(2):
                    psr = ps_r.tile([128, 128], f32, tag="psr")
                    for t in range(T):
                        for hl in range(4):
                            h = slab * 4 + hl
                            for sh in range(2):
                                base = t * 1024 + sh * 512
                                nc.tensor.matmul(
                                    psr[32 * hl:32 * (hl + 1), t * 32:(t + 1) * 32],
                                    lhsT=skv[:, base + h * 32: base + (h + 1) * 32],
                                    rhs=skv[:, base + 256 + h * 32: base + 256 + (h + 1) * 32],
                                    start=(sh == 0), stop=(sh == 1),
                                    tile_position=(0, 32 * hl),
                                )
                    dst = r_loc[:].rearrange("p (a w e) -> p a w e", a=8, w=8, e=32)[
                        :, slab * 4:(slab + 1) * 4, wi, :]
                    nc.vector.tensor_copy(dst, psr[:])
                if wi == 3:
                    exchange_half(0)
                elif wi == 7:
                    exchange_half(1)

            # r_all free layout: (slab2, t4, j32, e32)
            r_all_v = r_all[:].rearrange("p (a t j e) -> p a t j e", a=2, t=4, j=32, e=32)

            # ---- q-pass (feature-major) interleaved with gather emission ----
            def emit_q_window(wi):
                x_sb = x_tiles[wi]
                sqw = sq_w[wi]
                sqw_v = sqw[:].rearrange("p (f t e) -> p f t e", f=2, t=4, e=256)
                vq16h = st_p.tile([128, 512], f16, tag="vq")    # q v/2 state
                for t in range(T):
                    psq = ps_q.tile([128, 512], f32, tag="psq")
                    for fc in range(2):
                        reg = psq[:, fc * 256:(fc + 1) * 256]
                        for kc in range(2):
                            nc.tensor.matmul(
                                reg,
                                lhsT=wq_sb[:, kc * 256 + fc * 128:
                                           kc * 256 + (fc + 1) * 128],
                                rhs=x_sb[:, kc * 1024 + t * 256:
                                         kc * 1024 + (t + 1) * 256],
                                start=(kc == 0), stop=False)
                        nc.tensor.matmul(
                            reg, lhsT=bqh_c[:, fc * 128:(fc + 1) * 128],
                            rhs=ones1[:], start=False, stop=True)
                    hqps = st_p.tile([128, 512], f16, tag="hqps")
                    nc.scalar.activation(hqps[:], psq[:], ACT.Copy, bias=0.0, scale=1.0)
                    if t == 0:
                        hq = hqps
                    else:
                        hq = st_p.tile([128, 512], f16, tag="hq")
                        nc.vector.tensor_tensor(hq[:], hqps[:], vq16h[:], op=ALU.add)
                    sq_sl = sqw_v[:, :, t, :]
                    spike(SPK_Q[t], sq_sl, hq[:])
                    if t < T - 1:
                        ltq = st_p.tile([128, 512], f16, tag="ltq")
                        nc.vector.tensor_scalar(ltq[:], hq[:], 1.0, 0.5, ALU.is_lt, ALU.mult)
                        nc.vector.tensor_tensor(vq16h[:], hq[:], ltq[:], op=ALU.mult)

            # kv sums: routed gather baked per core, guarded by If on core id
            pid = None if sim_mode else nc.partition_id()

            def emit_gather(gather_windows):
                for r in range(NCORES):
                    if sim_mode and r != 0:
                        continue
                    b_of = r // 4
                    wg = r % 4
                    with (contextlib.nullcontext() if sim_mode else tc.If(pid == r)):
                        for wl in gather_windows:
                            wglob = wg * 8 + wl
                            idxs = [int(j) for j in routing_idx[b_of, wglob]]
                            dst = kv_w_all[:, wl * 256:(wl + 1) * 256]
                            # tree reduction, levels split across DVE and Pool
                            g1a = gt_p.tile([128, 256], f16, tag="g1a")
                            g1b = gt_p.tile([128, 256], f16, tag="g1b")
                            nc.vector.tensor_tensor(
                                g1a[:], r_all_v[:, :, :, idxs[0], :],
                                r_all_v[:, :, :, idxs[1], :], op=ALU.add)
                            nc.vector.tensor_tensor(
                                g1a[:], g1a[:],
                                r_all_v[:, :, :, idxs[2], :], op=ALU.add)
                            nc.vector.tensor_tensor(
                                g1a[:], g1a[:],
                                r_all_v[:, :, :, idxs[3], :], op=ALU.add)
                            g1c = gt_p.tile([128, 256], f16, tag="g1c")
                            nc.gpsimd.tensor_tensor(
                                g1b[:], r_all_v[:, :, :, idxs[4], :],
                                r_all_v[:, :, :, idxs[5], :], op=ALU.add)
                            nc.gpsimd.tensor_tensor(
                                g1c[:], r_all_v[:, :, :, idxs[6], :],
                                r_all_v[:, :, :, idxs[7], :], op=ALU.add)
                            nc.vector.tensor_tensor(g1a[:], g1a[:], g1b[:], op=ALU.add)
                            nc.vector.tensor_tensor(dst, g1a[:], g1c[:], op=ALU.add)

            for wi in range(NW):
                emit_q_window(wi)
                if wi % 2 == 1:
                    emit_gather([wi - 1, wi])
            ph1.close()
            ph2 = contextlib.ExitStack()
            ps_at = ph2.enter_context(tc.tile_pool(name="psat", bufs=4, space="PSUM"))
            ps_pj = ph2.enter_context(tc.tile_pool(name="pspj", bufs=4, space="PSUM"))

            # batched block-diagonal materialization: 2 batches x 4 DMAs
            kvw_v = kv_w_all[:].rearrange("p (w st e) -> p w st e", w=NW, st=8, e=32)
            bd_v = bd_all[:].rearrange("p (w st c) -> p w st c", w=NW, st=8, c=128)
            for hb in range(2):
                for hl in range(4):
                    nc.sync.dma_start(
                        bd_v[32 * hl:32 * (hl + 1), hb * 4:(hb + 1) * 4, :,
                             32 * hl:32 * (hl + 1)],
                        kvw_v[32 * hl:32 * (hl + 1), hb * 4:(hb + 1) * 4])

            # ============ segment 2: attention sweep, then proj sweep ========
            attn_tiles = []
            for wi in range(NW):
                bd = bd_all[:, wi * 1024:(wi + 1) * 1024]
                sqw = sq_w[wi]
                attn = xin_p.tile([128, 2 * NTOK], f16, tag="xsb")
                attn_tiles.append(attn)
                for th in range(2):
                    for slab in range(2):
                        psa = ps_at.tile([128, 512], f32, tag="psat")
                        for tl in range(2):
                            t = th * 2 + tl
                            st = slab * 4 + t
                            nc.tensor.matmul(
                                psa[:, tl * 256:(tl + 1) * 256],
                                lhsT=bd[:, st * 128:(st + 1) * 128],
                                rhs=sqw[:, slab * NTOK + t * 256:
                                        slab * NTOK + (t + 1) * 256],
                                start=True, stop=True,
                            )
                        dst_at = attn[:, slab * NTOK + th * 512:
                                      slab * NTOK + (th + 1) * 512]
                        ev = ATTN_EVAC[(wi * 4 + th * 2 + slab) % len(ATTN_EVAC)]
                        if ev == "a":
                            nc.scalar.activation(dst_at, psa[:], ACT.Copy,
                                                 bias=0.0, scale=1.0)
                        elif ev == "p":
                            nc.gpsimd.tensor_copy(dst_at, psa[:])
                        else:
                            nc.vector.tensor_copy(dst_at, psa[:])

            for wi in range(NW):
                attn = attn_tiles[wi]
                outsb = out_p.tile([128, 2 * NTOK], f32, tag="outsb")
                for cft in range(2):
                    for nch in range(2):
                        psp = ps_pj.tile([128, 512], f32, tag="pspj")
                        for kc in range(2):
                            nc.tensor.matmul(
                                psp[:],
                                lhsT=wp_sb[:, kc * 256 + cft * 128: kc * 256 + (cft + 1) * 128],
                                rhs=attn[:, kc * NTOK + nch * 512: kc * NTOK + (nch + 1) * 512],
                                start=(kc == 0), stop=(kc == 1),
                            )
                        nc.scalar.activation(
                            outsb[:, cft * NTOK + nch * 512: cft * NTOK + (nch + 1) * 512],
                            psp[:], ACT.Identity, bias=bp_sb[:, cft:cft + 1], scale=1.0)
                outd_v = out_d[wi].rearrange("c p (n f) -> c p n f", n=2, f=512)
                for cft in range(2):
                    for nch in range(2):
                        nc.sync.dma_start(
                            outd_v[cft, :, nch, :],
                            outsb[:, cft * NTOK + nch * 512: cft * NTOK + (nch + 1) * 512])
            ph2.close()

    _split_sync_waits(nc, mybir, maxw=1)
    return nc


def _host_prepost(x, w_qkv, b_qkv):
    """Window partition, routing."""
    xw = x.reshape(T, B, WT, GT, WH, GH, WW, GW, C) \
          .transpose(0, 1, 2, 4, 6, 3, 5, 7, 8).reshape(T, B, W, S, C)
    xbar = xw.mean(axis=(0, 3))                      # [B, W, C]
    q_reg = xbar @ w_qkv[:, :C] + b_qkv[:C]
    k_reg = xbar @ w_qkv[:, C:2 * C] + b_qkv[C:2 * C]
    a_r = np.einsum('bwc,bvc->bwv', q_reg, k_reg)
    routing_idx = np.argsort(-a_r, axis=-1)[:, :, :TOPK]   # [B, W, TOPK]
    return xw, routing_idx


def kernel(x, w_qkv, b_qkv, w_proj, b_proj):
    x = np.ascontiguousarray(np.asarray(x, dtype=np.float32))
    w_qkv = np.asarray(w_qkv, dtype=np.float32)
    b_qkv = np.asarray(b_qkv, dtype=np.float32)
    w_proj = np.asarray(w_proj, dtype=np.float32)
    b_proj = np.asarray(b_proj, dtype=np.float32)

    xw, routing_idx = _host_prepost(x, w_qkv, b_qkv)

    key = (routing_idx.tobytes(), SPK_KV, SPK_Q, ATTN_EVAC)
    if key not in _prog_cache:
        _prog_cache.clear()
        _prog_cache[key] = _build_program(routing_idx)
    nc = _prog_cache[key]

    # weights (shared across cores), pre-halved for the LIF /TAU
    wkv_arr = (0.5 * w_qkv[:, C:]).reshape(2, 128, 512).astype(np.float16)
    wq_arr = (0.5 * w_qkv[:, :C]).reshape(2, 128, 256).astype(np.float16)
    bkvh_row = np.tile((0.25 * b_qkv[C:]).astype(np.float16), 2).reshape(1, 1024)
    bkvh_arr = np.concatenate([bkvh_row, bkvh_row], axis=0)          # [2,1024]
    bqh_row = (0.25 * b_qkv[:C]).astype(np.float16).reshape(1, 256)
    bqh_arr = np.concatenate([bqh_row, bqh_row], axis=0)             # [2,256]
    wp = (SCALE * w_proj).reshape(2, 128, 256).astype(np.float16)
    bp = b_proj.reshape(2, 128, 1).astype(np.float32)

    in_maps = []
    for r in range(NCORES):
        b_of, wg = r // 4, r % 4
        xwc = xw[:, b_of, wg * 8:(wg + 1) * 8]              # [T, 8, S, C]
        xl = np.ascontiguousarray(
            xwc.transpose(1, 3, 0, 2).reshape(NW, 2, 128, NTOK)).astype(np.float16)
        in_maps.append({
            "x_in": xl,
            "wkv_in": wkv_arr, "wq_in": wq_arr,
            "bkvh_in": bkvh_arr, "bqh_in": bqh_arr,
            "wp_in": wp, "bp_in": bp,
        })

    from concourse.bass_utils import run_bass_kernel_spmd
    res = run_bass_kernel_spmd(nc, in_maps, core_ids=list(range(NCORES)))

    # assemble output
    yw = np.empty((T, B, W, S, C), dtype=np.float32)
    for r in range(NCORES):
        b_of, wg = r // 4, r % 4
        o = res.results[r]["out_d"]                          # [NW, 2, 128, NTOK]
        o = o.reshape(NW, 2, 128, T, S).transpose(0, 3, 4, 1, 2).reshape(NW, T, S, C)
        for wl in range(NW):
            yw[:, b_of, wg * 8 + wl] = o[wl]

    y = yw.reshape(T, B, WT, WH, WW, GT, GH, GW, C) \
          .transpose(0, 1, 2, 5, 3, 6, 4, 7, 8).reshape(T, B, Lt, Lh, Lw, C)
    return y


# revision 47
# speedup vs baseline: 1.9573x; 1.0160x over previous
"""BiLevelRoutingAttention Trainium2 kernel (8-core SPMD).

Sharding: core r handles batch b = r//4 and windows w in [ (r%4)*8, (r%4)*8+8 ).
Routing (region top-k) is computed on host via linearity of the mean:
    q_region = mean_{t,s}(xw) @ Wq + bq  (exact up to fp reassociation).

Single-pass fp16 qkv matmuls (rel-err budget 2e-2; this lands ~3e-3).
LIF per timestep (TAU=2, pre-halved weights so ps = x@W/2 + b/2):
    hps = Act.Copy(ps)            (PSUM evacuation to fp16 SBUF)
    h   = hps + v/2               (DVE fp16 TT, v/2 kept as state)
    s   = step(h)                 (engine per t: Pool is_ge exact /
                                   Act sigmoid(sigma*(h-1)) / DVE is_ge fp16)
    v/2 = h*(h<1)*0.5             (DVE TS + TT, fp16 fast modes)
Biases enter via tiny ones-row matmuls accumulated into PSUM (K=2).
Spikes are binary -> attention arithmetic is exact integer math in fp16.

Structure: [kv pass + R for all 8 windows, with the R AllGather split in
two halves overlapped into phase 1] -> [q pass overlapping the exchange,
with per-window routed-gather trees (DVE+Pool) interleaved] -> [batched
block-diagonal kv materialization via 8 DMAs] -> [attention sweep: one
256-row matmul per (slab,t) against the BD stationary] -> [proj sweep].
Engine assignment knobs: KSPK/KSPKQ (spike engine per t: a/p/d), KAEV
(attn evacuation engines).
"""
import numpy as np
import os as _os

# ---- problem constants (hardcoded per contract) ----
T, B, Lt, Lh, Lw, C = 4, 2, 8, 32, 32, 256
WT, WH, WW = 2, 4, 4
W = WT * WH * WW            # 32 windows
GT, GH, GW = Lt // WT, Lh // WH, Lw // WW
S = GT * GH * GW            # 256 tokens per window
H, D = 8, C // 8            # 8 heads, 32 dim
TOPK = 8
SCALE = float(D) ** -0.5
NCORES = 8
NW = 8                      # windows per core
NTOK = T * S                # 1024 token-instances per window

SPK_KV = _os.environ.get("KSPK", "pdad")    # per-t engine: a=act p=pool d=dve
SPK_Q = _os.environ.get("KSPKQ", "appp")
ATTN_EVAC = _os.environ.get("KAEV", "dddd")
REVAC = _os.environ.get("KREV", "a")  # act | dve | pool
SIGMA = 30000.0                              # sigmoid step sharpness

_prog_cache = {}


def _split_sync_waits(nc, mybir, maxw=1):
    """walrus in this container rejects >1 sync wait per instruction; split
    excess waits onto NoOp instructions inserted just before."""
    for bb in nc.main_func.blocks:
        new_list = []
        for ins in bb.instructions:
            si = ins.sync_info
            waits = list(si.on_wait) if si is not None and si.on_wait else []
            if len(waits) > maxw:
                extra = waits[:-maxw]
                keep = waits[-maxw:]
                idx = 0
                while extra:
                    chunk, extra = extra[:maxw], extra[maxw:]
                    nop = mybir.InstNoOp(name=f"{ins.name}-wsplit{idx}", ins=[], outs=[])
                    nop.engine = ins.engine
                    nop.sync_info = mybir.SyncInfo(on_wait=chunk, on_update=[])
                    new_list.append(nop)
                    idx += 1
                ins.sync_info = mybir.SyncInfo(
                    on_wait=keep,
                    on_update=list(si.on_update) if si.on_update else [],
                )
            new_list.append(ins)
        bb.instructions[:] = new_list


def _build_program(routing_idx, sim_mode=False):
    """routing_idx: [B, W, TOPK] int array (host-computed). Returns nc.
    sim_mode: no collective / no If-chain (single-core TimelineSim)."""
    import contextlib
    import concourse.bass as bass
    import concourse.mybir as mybir
    import concourse.tile as tile

    f32 = mybir.dt.float32
    f16 = mybir.dt.float16
    ALU = mybir.AluOpType
    ACT = mybir.ActivationFunctionType

    nc = bass.Bass(num_devices=NCORES)

    # ---- I/O ----
    # x feature-major: [wi, kc, c(128), (t,s)]
    x_in = nc.dram_tensor("x_in", [NW, 2, 128, NTOK], f16, kind="ExternalInput")
    wkv_in = nc.dram_tensor("wkv_in", [2, 128, 512], f16, kind="ExternalInput")
    wq_in = nc.dram_tensor("wq_in", [2, 128, 256], f16, kind="ExternalInput")
    bkvh_in = nc.dram_tensor("bkvh_in", [1, 2048], mybir.dt.float8e4, kind="ExternalInput")
    bqh_in = nc.dram_tensor("bqh_in", [1, 512], mybir.dt.float8e4, kind="ExternalInput")
    wp_in = nc.dram_tensor("wp_in", [2, 128, 256], f16, kind="ExternalInput")
    bp_in = nc.dram_tensor("bp_in", [2, 128, 1], f32, kind="ExternalInput")
    out_d = nc.dram_tensor("out_d", [NW, 2, 128, NTOK], f32, kind="ExternalOutput")

    with tile.TileContext(nc) as tc:
        with (
            tc.tile_pool(name="const", bufs=1) as constp,
            tc.tile_pool(name="xin", bufs=8) as xin_p,
            tc.tile_pool(name="skv", bufs=2) as skv_p,
            tc.tile_pool(name="state", bufs=3) as st_p,
            tc.tile_pool(name="persist", bufs=1) as pers_p,
            tc.tile_pool(name="outs", bufs=2) as out_p,
            tc.tile_pool(name="gtmp", bufs=2) as gt_p,
            tc.tile_pool(name="dram", bufs=1, space="DRAM") as dram_p,
        ):
            # ---- constants / weights (kv-critical ones first; rest are
            # DMA'd after the first x tiles to keep the PE fed early) ----
            wkv_sb = constp.tile([128, 2 * 512], f16)
            nc.sync.dma_start(wkv_sb[:, 0:512], wkv_in[0])
            bkvh_c = constp.tile([1, 2048], mybir.dt.float8e4)  # (kt2, 1024)
            wq_sb = constp.tile([128, 2 * 256], f16)
            wp_sb = constp.tile([128, 2 * 256], f16)
            bp_sb = constp.tile([128, 2], f32)
            bqh_c = constp.tile([1, 512], mybir.dt.float8e4)   # (kt2, 256)
            ones1 = constp.tile([1, 512], mybir.dt.float8e4)
            nc.vector.memset(ones1[:], 1.0)
            nsig = constp.tile([128, 1], f32)
            nc.vector.memset(nsig[:], -SIGMA)

            def load_late_consts():
                nc.sync.dma_start(
                    wq_sb[:].rearrange("p (k f) -> p k f", k=2, f=256),
                    wq_in[:].rearrange("k p f -> p k f"))
                nc.sync.dma_start(
                    wp_sb[:].rearrange("p (k f) -> p k f", k=2, f=256),
                    wp_in[:].rearrange("k p f -> p k f"))
                nc.sync.dma_start(
                    bp_sb[:].rearrange("p (k o) -> p k o", k=2, o=1),
                    bp_in[:].rearrange("k p o -> p k o"))
                nc.sync.dma_start(bqh_c[:], bqh_in[:])

            # persistent across phases
            r_loc = pers_p.tile([128, 2048], f16)          # local R, (slab,t,wi,e)
            r_all = pers_p.tile([128, 8192], f16)          # gathered R, (slab,t,j,e)
            kv_w_all = pers_p.tile([128, NW * 256], f16)
            sq_w = [pers_p.tile([128, 2048], f16, name=f"sqw{i}") for i in range(NW)]
            # block-diagonal kv for all windows; off-diagonal zeros are set
            # once and persist (diag blocks land at fixed offsets).
            bd_all = pers_p.tile([128, NW * 1024], f16)
            nc.gpsimd.memset(bd_all[:], 0.0)

            def spike(eng, dst, src):
                if eng == 'a':
                    nc.scalar.activation(dst, src, ACT.Sigmoid,
                                         bias=nsig[:, 0:1], scale=SIGMA)
                elif eng == 'p':
                    nc.gpsimd.tensor_scalar(dst, src, 1.0, None, ALU.is_ge)
                else:
                    nc.vector.tensor_scalar(dst, src, 1.0, None, ALU.is_ge)

            rb_inA = dram_p.tile([128, 1024], f16)
            rb_outA = dram_p.tile([4, 128, 1024], f16)
            rb_inB = dram_p.tile([128, 1024], f16)
            rb_outB = dram_p.tile([4, 128, 1024], f16)

            def exchange_half(half):
                rb_i, rb_o = (rb_inA, rb_outA) if half == 0 else (rb_inB, rb_outB)
                r_loc_v = r_loc[:].rearrange("p (a w e) -> p a w e", a=8, w=8, e=32)
                nc.sync.dma_start(
                    rb_i[:].rearrange("p (a w e) -> p a w e", a=8, w=4, e=32),
                    r_loc_v[:, :, half * 4:(half + 1) * 4, :])
                if sim_mode:
                    for rk in range(4):
                        nc.sync.dma_start(rb_o[rk], rb_i[:])
                else:
                    nc.gpsimd.collective_compute(
                        "AllGather",
                        mybir.AluOpType.bypass,
                        replica_groups=[[0, 1, 2, 3], [4, 5, 6, 7]],
                        ins=[rb_i[:].opt()],
                        outs=[rb_o[:].opt()],
                    )
                r_all_vv = r_all[:].rearrange("p (a t j e) -> p a t j e", a=2, t=4, j=32, e=32)
                for rk in range(4):
                    srcv = rb_o[rk].rearrange("p (a w e) -> p a w e", a=8, w=4, e=32)                         .rearrange("p (s t) w e -> p s t w e", s=2, t=4)
                    nc.sync.dma_start(
                        r_all_vv[:, :, :, rk * 8 + half * 4: rk * 8 + (half + 1) * 4, :],
                        srcv)

            # ================= phase 1: qkv + LIF + R =================
            ph1 = contextlib.ExitStack()
            ps_a = ph1.enter_context(tc.tile_pool(name="psA", bufs=2, space="PSUM"))
            ps_q = ph1.enter_context(tc.tile_pool(name="psQ", bufs=2, space="PSUM"))
            ps_r = ph1.enter_context(tc.tile_pool(name="psr", bufs=2, space="PSUM"))
            x_tiles = []
            for wi in range(NW):
                x_sb = xin_p.tile([128, 2048], f16, tag="xsb")
                x_tiles.append(x_sb)
                for half in range(2):
                    for kc in range(2):
                        nc.sync.dma_start(
                            x_sb[:, kc * 1024 + half * 512: kc * 1024 + (half + 1) * 512],
                            x_in[wi, kc, :, half * 512:(half + 1) * 512])

                if wi == 0:
                    nc.sync.dma_start(wkv_sb[:, 512:1024], wkv_in[1])
                    nc.sync.dma_start(bkvh_c[:], bkvh_in[:])
                elif wi == 1:
                    load_late_consts()
                skv = skv_p.tile([128, 4096], f16, tag="skv")
                v16h = st_p.tile([128, 1024], f16, tag="vkv")   # kv v/2 state
                for t in range(T):
                    # ---- kv matmuls (token-major); bias via ones-row mm ----
                    ps = ps_a.tile([128, 1024], f32, tag="psa")
                    for sh in range(2):
                        reg = ps[:, sh * 512:(sh + 1) * 512]
                        for kc in range(2):
                            nc.tensor.matmul(
                                reg,
                                lhsT=x_sb[:, kc * 1024 + t * 256 + sh * 128:
                                          kc * 1024 + t * 256 + (sh + 1) * 128],
                                rhs=wkv_sb[:, kc * 512:(kc + 1) * 512],
                                start=(kc == 0), stop=False)
                        nc.tensor.matmul(
                            reg,
                            lhsT=ones1[:].rearrange("o (k f) -> o k f", k=2, f=256)[
                                :, :, :128],
                            rhs=bkvh_c[:].rearrange("o (k f) -> o k f", k=2, f=1024)[
                                :, :, sh * 512:(sh + 1) * 512],
                            start=False, stop=True,
                            perf_mode=mybir.MatmulPerfMode.DoubleRow)
                    # evacuate h-partial = x@W/2 + b/2 (Act); h = hps + v/2
                    hps = st_p.tile([128, 1024], f16, tag="hps")
                    nc.scalar.activation(hps[:], ps[:], ACT.Copy, bias=0.0, scale=1.0)
                    if t == 0:
                        hkv = hps
                    else:
                        hkv = st_p.tile([128, 1024], f16, tag="hkv")
                        nc.vector.tensor_tensor(hkv[:], hps[:], v16h[:], op=ALU.add)
                    s_sl = skv[:, t * 1024:(t + 1) * 1024]
                    spike(SPK_KV[t], s_sl, hkv[:])
                    if t < T - 1:
                        lt = st_p.tile([128, 1024], f16, tag="lt")
                        nc.vector.tensor_scalar(lt[:], hkv[:], 1.0, 0.5, ALU.is_lt, ALU.mult)
                        nc.vector.tensor_tensor(v16h[:], hkv[:], lt[:], op=ALU.mult)

                # ---- R = k^T v per (t, head): col-tiled 4 heads ----
                for slab in range(2):
                    psr = ps_r.tile([128, 128], f32, tag="psr")
                    for t in range(T):
                        for hl in range(4):
                            h = slab * 4 + hl
                            for sh in range(2):
                                base = t * 1024 + sh * 512
                                nc.tensor.matmul(
                                    psr[32 * hl:32 * (hl + 1), t * 32:(t + 1) * 32],
                                    lhsT=skv[:, base + h * 32: base + (h + 1) * 32],
                                    rhs=skv[:, base + 256 + h * 32: base + 256 + (h + 1) * 32],
                                    start=(sh == 0), stop=(sh == 1),
                                    tile_position=(0, 32 * hl),
                                )
                    dst = r_loc[:].rearrange("p (a w e) -> p a w e", a=8, w=8, e=32)[
                        :, slab * 4:(slab + 1) * 4, wi, :]
                    if REVAC == "a":
                        nc.scalar.activation(dst, psr[:], ACT.Copy, bias=0.0, scale=1.0)
                    elif REVAC == "p":
                        nc.gpsimd.tensor_copy(dst, psr[:])
                    else:
                        nc.vector.tensor_copy(dst, psr[:])
                if wi == 3:
                    exchange_half(0)
                elif wi == 7:
                    exchange_half(1)

            # r_all free layout: (slab2, t4, j32, e32)
            r_all_v = r_all[:].rearrange("p (a t j e) -> p a t j e", a=2, t=4, j=32, e=32)

            # ---- q-pass (feature-major) interleaved with gather emission ----
            def emit_q_window(wi):
                x_sb = x_tiles[wi]
                sqw = sq_w[wi]
                sqw_v = sqw[:].rearrange("p (f t e) -> p f t e", f=2, t=4, e=256)
                vq16h = st_p.tile([128, 512], f16, tag="vq")    # q v/2 state
                for t in range(T):
                    psq = ps_q.tile([128, 512], f32, tag="psq")
                    for fc in range(2):
                        reg = psq[:, fc * 256:(fc + 1) * 256]
                        for kc in range(2):
                            nc.tensor.matmul(
                                reg,
                                lhsT=wq_sb[:, kc * 256 + fc * 128:
                                           kc * 256 + (fc + 1) * 128],
                                rhs=x_sb[:, kc * 1024 + t * 256:
                                         kc * 1024 + (t + 1) * 256],
                                start=(kc == 0), stop=False)
                        nc.tensor.matmul(
                            reg,
                            lhsT=bqh_c[:].rearrange("o (k f) -> o k f", k=2, f=256)[
                                :, :, fc * 128:(fc + 1) * 128],
                            rhs=ones1[:].rearrange("o (k f) -> o k f", k=2, f=256),
                            start=False, stop=True,
                            perf_mode=mybir.MatmulPerfMode.DoubleRow)
                    hqps = st_p.tile([128, 512], f16, tag="hqps")
                    nc.scalar.activation(hqps[:], psq[:], ACT.Copy, bias=0.0, scale=1.0)
                    if t == 0:
                        hq = hqps
                    else:
                        hq = st_p.tile([128, 512], f16, tag="hq")
                        nc.vector.tensor_tensor(hq[:], hqps[:], vq16h[:], op=ALU.add)
                    sq_sl = sqw_v[:, :, t, :]
                    spike(SPK_Q[t], sq_sl, hq[:])
                    if t < T - 1:
                        ltq = st_p.tile([128, 512], f16, tag="ltq")
                        nc.vector.tensor_scalar(ltq[:], hq[:], 1.0, 0.5, ALU.is_lt, ALU.mult)
                        nc.vector.tensor_tensor(vq16h[:], hq[:], ltq[:], op=ALU.mult)

            kvw_v = kv_w_all[:].rearrange("p (w st e) -> p w st e", w=NW, st=8, e=32)
            bd_v = bd_all[:].rearrange("p (w st c) -> p w st c", w=NW, st=8, c=128)
            # kv sums: routed gather baked per core, guarded by If on core id
            pid = None if sim_mode else nc.partition_id()

            def emit_gather(gather_windows):
                for r in range(NCORES):
                    if sim_mode and r != 0:
                        continue
                    b_of = r // 4
                    wg = r % 4
                    with (contextlib.nullcontext() if sim_mode else tc.If(pid == r)):
                        for wl in gather_windows:
                            wglob = wg * 8 + wl
                            idxs = [int(j) for j in routing_idx[b_of, wglob]]
                            dst = kv_w_all[:, wl * 256:(wl + 1) * 256]
                            # tree reduction, levels split across DVE and Pool
                            g1a = gt_p.tile([128, 256], f16, tag="g1a")
                            g1b = gt_p.tile([128, 256], f16, tag="g1b")
                            nc.vector.tensor_tensor(
                                g1a[:], r_all_v[:, :, :, idxs[0], :],
                                r_all_v[:, :, :, idxs[1], :], op=ALU.add)
                            nc.vector.tensor_tensor(
                                g1a[:], g1a[:],
                                r_all_v[:, :, :, idxs[2], :], op=ALU.add)
                            nc.vector.tensor_tensor(
                                g1a[:], g1a[:],
                                r_all_v[:, :, :, idxs[3], :], op=ALU.add)
                            g1c = gt_p.tile([128, 256], f16, tag="g1c")
                            nc.gpsimd.tensor_tensor(
                                g1b[:], r_all_v[:, :, :, idxs[4], :],
                                r_all_v[:, :, :, idxs[5], :], op=ALU.add)
                            nc.gpsimd.tensor_tensor(
                                g1c[:], r_all_v[:, :, :, idxs[6], :],
                                r_all_v[:, :, :, idxs[7], :], op=ALU.add)
                            nc.vector.tensor_tensor(g1a[:], g1a[:], g1b[:], op=ALU.add)
                            nc.vector.tensor_tensor(dst, g1a[:], g1c[:], op=ALU.add)

            for wi in range(NW):
                emit_q_window(wi)
                if wi % 2 == 1:
                    emit_gather([wi - 1, wi])
                if wi in (3, 7):
                    hb = wi // 4
                    for hl in range(4):
                        nc.sync.dma_start(
                            bd_v[32 * hl:32 * (hl + 1), hb * 4:(hb + 1) * 4, :,
                                 32 * hl:32 * (hl + 1)],
                            kvw_v[32 * hl:32 * (hl + 1), hb * 4:(hb + 1) * 4])
            ph1.close()
            ph2 = contextlib.ExitStack()
            ps_at = ph2.enter_context(tc.tile_pool(name="psat", bufs=4, space="PSUM"))
            ps_pj = ph2.enter_context(tc.tile_pool(name="pspj", bufs=4, space="PSUM"))


            # ============ segment 2: attn/proj interleaved (lag 2) ===========
            attn_tiles = {}

            def emit_attn(wi):
                bd = bd_all[:, wi * 1024:(wi + 1) * 1024]
                sqw = sq_w[wi]
                attn = xin_p.tile([128, 2 * NTOK], f16, tag="xsb")
                attn_tiles[wi] = attn
                for th in range(2):
                    for slab in range(2):
                        psa = ps_at.tile([128, 512], f32, tag="psat")
                        for tl in range(2):
                            t = th * 2 + tl
                            st = slab * 4 + t
                            nc.tensor.matmul(
                                psa[:, tl * 256:(tl + 1) * 256],
                                lhsT=bd[:, st * 128:(st + 1) * 128],
                                rhs=sqw[:, slab * NTOK + t * 256:
                                        slab * NTOK + (t + 1) * 256],
                                start=True, stop=True,
                            )
                        dst_at = attn[:, slab * NTOK + th * 512:
                                      slab * NTOK + (th + 1) * 512]
                        ev = ATTN_EVAC[(wi * 4 + th * 2 + slab) % len(ATTN_EVAC)]
                        if ev == "a":
                            nc.scalar.activation(dst_at, psa[:], ACT.Copy,
                                                 bias=0.0, scale=1.0)
                        elif ev == "p":
                            nc.gpsimd.tensor_copy(dst_at, psa[:])
                        else:
                            nc.vector.tensor_copy(dst_at, psa[:])

            def emit_proj(wi):
                attn = attn_tiles[wi]
                outsb = out_p.tile([128, 2 * NTOK], f32, tag="outsb")
                outd_v = out_d[wi].rearrange("c p (n f) -> c p n f", n=2, f=512)
                for cft in range(2):
                    for nch in range(2):
                        psp = ps_pj.tile([128, 512], f32, tag="pspj")
                        for kc in range(2):
                            nc.tensor.matmul(
                                psp[:],
                                lhsT=wp_sb[:, kc * 256 + cft * 128: kc * 256 + (cft + 1) * 128],
                                rhs=attn[:, kc * NTOK + nch * 512: kc * NTOK + (nch + 1) * 512],
                                start=(kc == 0), stop=(kc == 1),
                            )
                        nc.scalar.activation(
                            outsb[:, cft * NTOK + nch * 512: cft * NTOK + (nch + 1) * 512],
                            psp[:], ACT.Identity, bias=bp_sb[:, cft:cft + 1], scale=1.0)
                        nc.sync.dma_start(
                            outd_v[cft, :, nch, :],
                            outsb[:, cft * NTOK + nch * 512: cft * NTOK + (nch + 1) * 512])

            for wi in range(NW):
                emit_attn(wi)
                if wi >= 2:
                    emit_proj(wi - 2)
            emit_proj(NW - 2)
            emit_proj(NW - 1)
            ph2.close()

    _split_sync_waits(nc, mybir, maxw=1)
    return nc


def _host_prepost(x, w_qkv, b_qkv):
    """Window partition, routing."""
    xw = x.reshape(T, B, WT, GT, WH, GH, WW, GW, C) \
          .transpose(0, 1, 2, 4, 6, 3, 5, 7, 8).reshape(T, B, W, S, C)
    xbar = xw.mean(axis=(0, 3))                      # [B, W, C]
    q_reg = xbar @ w_qkv[:, :C] + b_qkv[:C]
    k_reg = xbar @ w_qkv[:, C:2 * C] + b_qkv[C:2 * C]
    a_r = np.einsum('bwc,bvc->bwv', q_reg, k_reg)
    routing_idx = np.argsort(-a_r, axis=-1)[:, :, :TOPK]   # [B, W, TOPK]
    return xw, routing_idx


def kernel(x, w_qkv, b_qkv, w_proj, b_proj):
    x = np.ascontiguousarray(np.asarray(x, dtype=np.float32))
    w_qkv = np.asarray(w_qkv, dtype=np.float32)
    b_qkv = np.asarray(b_qkv, dtype=np.float32)
    w_proj = np.asarray(w_proj, dtype=np.float32)
    b_proj = np.asarray(b_proj, dtype=np.float32)

    xw, routing_idx = _host_prepost(x, w_qkv, b_qkv)

    key = (routing_idx.tobytes(), SPK_KV, SPK_Q, ATTN_EVAC, REVAC)
    if key not in _prog_cache:
        _prog_cache.clear()
        _prog_cache[key] = _build_program(routing_idx)
    nc = _prog_cache[key]

    # weights (shared across cores), pre-halved for the LIF /TAU
    wkv_arr = (0.5 * w_qkv[:, C:]).reshape(2, 128, 512).astype(np.float16)
    wq_arr = (0.5 * w_qkv[:, :C]).reshape(2, 128, 256).astype(np.float16)
    import ml_dtypes
    f8 = ml_dtypes.float8_e4m3fn
    bkvh_row = np.tile((0.25 * b_qkv[C:]).astype(f8), 2).reshape(1024)
    bkvh_arr = np.concatenate([bkvh_row, bkvh_row]).reshape(1, 2048)  # kt-major
    bqh_row = (0.25 * b_qkv[:C]).astype(f8).reshape(256)
    bqh_arr = np.concatenate([bqh_row, bqh_row]).reshape(1, 512)      # kt-major
    wp = (SCALE * w_proj).reshape(2, 128, 256).astype(np.float16)
    bp = b_proj.reshape(2, 128, 1).astype(np.float32)

    in_maps = []
    for r in range(NCORES):
        b_of, wg = r // 4, r % 4
        xwc = xw[:, b_of, wg * 8:(wg + 1) * 8]              # [T, 8, S, C]
        xl = np.ascontiguousarray(
            xwc.transpose(1, 3, 0, 2).reshape(NW, 2, 128, NTOK)).astype(np.float16)
        in_maps.append({
            "x_in": xl,
            "wkv_in": wkv_arr, "wq_in": wq_arr,
            "bkvh_in": bkvh_arr, "bqh_in": bqh_arr,
            "wp_in": wp, "bp_in": bp,
        })

    from concourse.bass_utils import run_bass_kernel_spmd
    res = run_bass_kernel_spmd(nc, in_maps, core_ids=list(range(NCORES)))

    # assemble output
    yw = np.empty((T, B, W, S, C), dtype=np.float32)
    for r in range(NCORES):
        b_of, wg = r // 4, r % 4
        o = res.results[r]["out_d"]                          # [NW, 2, 128, NTOK]
        o = o.reshape(NW, 2, 128, T, S).transpose(0, 3, 4, 1, 2).reshape(NW, T, S, C)
        for wl in range(NW):
            yw[:, b_of, wg * 8 + wl] = o[wl]

    y = yw.reshape(T, B, WT, WH, WW, GT, GH, GW, C) \
          .transpose(0, 1, 2, 5, 3, 6, 4, 7, 8).reshape(T, B, Lt, Lh, Lw, C)
    return y


# revision 49
# speedup vs baseline: 1.9820x; 1.0127x over previous
"""BiLevelRoutingAttention Trainium2 kernel (8-core SPMD).

Sharding: core r handles batch b = r//4 and windows w in [ (r%4)*8, (r%4)*8+8 ).
Routing (region top-k) is computed on host via linearity of the mean:
    q_region = mean_{t,s}(xw) @ Wq + bq  (exact up to fp reassociation).

Single-pass fp16 qkv matmuls (rel-err budget 2e-2; this lands ~3e-3).
LIF per timestep (TAU=2, pre-halved weights so ps = x@W/2 + b/2):
    hps = Act.Copy(ps)            (PSUM evacuation to fp16 SBUF)
    h   = hps + v/2               (DVE fp16 TT, v/2 kept as state)
    s   = step(h)                 (engine per t: Pool is_ge exact /
                                   Act sigmoid(sigma*(h-1)) / DVE is_ge fp16)
    v/2 = h*(h<1)*0.5             (DVE TS + TT, fp16 fast modes)
Biases enter via tiny ones-row matmuls accumulated into PSUM (K=2).
Spikes are binary -> attention arithmetic is exact integer math in fp16.

Structure: [kv pass + R for all 8 windows, with the R AllGather split in
two halves overlapped into phase 1] -> [q pass overlapping the exchange,
with per-window routed-gather trees (DVE+Pool) interleaved] -> [batched
block-diagonal kv materialization via 8 DMAs] -> [attention sweep: one
256-row matmul per (slab,t) against the BD stationary] -> [proj sweep].
Engine assignment knobs: KSPK/KSPKQ (spike engine per t: a/p/d), KAEV
(attn evacuation engines).
"""
import numpy as np
import os as _os

# ---- problem constants (hardcoded per contract) ----
T, B, Lt, Lh, Lw, C = 4, 2, 8, 32, 32, 256
WT, WH, WW = 2, 4, 4
W = WT * WH * WW            # 32 windows
GT, GH, GW = Lt // WT, Lh // WH, Lw // WW
S = GT * GH * GW            # 256 tokens per window
H, D = 8, C // 8            # 8 heads, 32 dim
TOPK = 8
SCALE = float(D) ** -0.5
NCORES = 8
NW = 8                      # windows per core
NTOK = T * S                # 1024 token-instances per window

SPK_KV = _os.environ.get("KSPK", "pdad")    # per-t engine: a=act p=pool d=dve
SPK_Q = _os.environ.get("KSPKQ", "appp")
ATTN_EVAC = _os.environ.get("KAEV", "dddd")
REVAC = _os.environ.get("KREV", "d")  # act | dve | pool
SIGMA = 30000.0                              # sigmoid step sharpness

_prog_cache = {}


def _split_sync_waits(nc, mybir, maxw=1):
    """walrus in this container rejects >1 sync wait per instruction; split
    excess waits onto NoOp instructions inserted just before."""
    for bb in nc.main_func.blocks:
        new_list = []
        for ins in bb.instructions:
            si = ins.sync_info
            waits = list(si.on_wait) if si is not None and si.on_wait else []
            if len(waits) > maxw:
                extra = waits[:-maxw]
                keep = waits[-maxw:]
                idx = 0
                while extra:
                    chunk, extra = extra[:maxw], extra[maxw:]
                    nop = mybir.InstNoOp(name=f"{ins.name}-wsplit{idx}", ins=[], outs=[])
                    nop.engine = ins.engine
                    nop.sync_info = mybir.SyncInfo(on_wait=chunk, on_update=[])
                    new_list.append(nop)
                    idx += 1
                ins.sync_info = mybir.SyncInfo(
                    on_wait=keep,
                    on_update=list(si.on_update) if si.on_update else [],
                )
            new_list.append(ins)
        bb.instructions[:] = new_list


def _build_program(routing_idx, sim_mode=False):
    """routing_idx: [B, W, TOPK] int array (host-computed). Returns nc.
    sim_mode: no collective / no If-chain (single-core TimelineSim)."""
    import contextlib
    import concourse.bass as bass
    import concourse.mybir as mybir
    import concourse.tile as tile

    f32 = mybir.dt.float32
    f16 = mybir.dt.float16
    ALU = mybir.AluOpType
    ACT = mybir.ActivationFunctionType

    nc = bass.Bass(num_devices=NCORES)

    # ---- I/O ----
    # x feature-major: [wi, kc, c(128), (t,s)]
    x_in = nc.dram_tensor("x_in", [NW, 2, 128, NTOK], f16, kind="ExternalInput")
    wkv_in = nc.dram_tensor("wkv_in", [2, 128, 512], f16, kind="ExternalInput")
    wq_in = nc.dram_tensor("wq_in", [2, 128, 256], f16, kind="ExternalInput")
    bkvh_in = nc.dram_tensor("bkvh_in", [1, 2048], mybir.dt.float8e4, kind="ExternalInput")
    bqh_in = nc.dram_tensor("bqh_in", [1, 512], mybir.dt.float8e4, kind="ExternalInput")
    wp_in = nc.dram_tensor("wp_in", [2, 128, 256], f16, kind="ExternalInput")
    bp_in = nc.dram_tensor("bp_in", [2, 128, 1], f32, kind="ExternalInput")
    out_d = nc.dram_tensor("out_d", [NW, 2, 128, NTOK], f32, kind="ExternalOutput")

    with tile.TileContext(nc) as tc:
        with (
            tc.tile_pool(name="const", bufs=1) as constp,
            tc.tile_pool(name="xin", bufs=8) as xin_p,
            tc.tile_pool(name="skv", bufs=2) as skv_p,
            tc.tile_pool(name="state", bufs=3) as st_p,
            tc.tile_pool(name="persist", bufs=1) as pers_p,
            tc.tile_pool(name="outs", bufs=2) as out_p,
            tc.tile_pool(name="gtmp", bufs=2) as gt_p,
            tc.tile_pool(name="dram", bufs=1, space="DRAM") as dram_p,
        ):
            # ---- constants / weights (kv-critical ones first; rest are
            # DMA'd after the first x tiles to keep the PE fed early) ----
            wkv_sb = constp.tile([128, 2 * 512], f16)
            nc.sync.dma_start(wkv_sb[:, 0:512], wkv_in[0])
            bkvh_c = constp.tile([1, 2048], mybir.dt.float8e4)  # (kt2, 1024)
            wq_sb = constp.tile([128, 2 * 256], f16)
            wp_sb = constp.tile([128, 2 * 256], f16)
            bp_sb = constp.tile([128, 2], f32)
            bqh_c = constp.tile([1, 512], mybir.dt.float8e4)   # (kt2, 256)
            ones1 = constp.tile([1, 512], mybir.dt.float8e4)
            nc.vector.memset(ones1[:], 1.0)
            nsig = constp.tile([128, 1], f32)
            nc.vector.memset(nsig[:], -SIGMA)

            def load_late_consts():
                nc.sync.dma_start(
                    wq_sb[:].rearrange("p (k f) -> p k f", k=2, f=256),
                    wq_in[:].rearrange("k p f -> p k f"))
                nc.sync.dma_start(
                    wp_sb[:].rearrange("p (k f) -> p k f", k=2, f=256),
                    wp_in[:].rearrange("k p f -> p k f"))
                nc.sync.dma_start(
                    bp_sb[:].rearrange("p (k o) -> p k o", k=2, o=1),
                    bp_in[:].rearrange("k p o -> p k o"))
                nc.sync.dma_start(bqh_c[:], bqh_in[:])

            # persistent across phases
            r_loc = pers_p.tile([128, 2048], f16)          # local R, (slab,t,wi,e)
            r_all = pers_p.tile([128, 8192], f16)          # gathered R, (slab,t,j,e)
            kv_w_all = pers_p.tile([128, NW * 256], f16)
            sq_w = [pers_p.tile([128, 2048], f16, name=f"sqw{i}") for i in range(NW)]
            # block-diagonal kv for all windows; off-diagonal zeros are set
            # once and persist (diag blocks land at fixed offsets).
            bd_all = pers_p.tile([128, NW * 1024], f16)
            nc.gpsimd.memset(bd_all[:], 0.0)

            def spike(eng, dst, src):
                if eng == 'a':
                    nc.scalar.activation(dst, src, ACT.Sigmoid,
                                         bias=nsig[:, 0:1], scale=SIGMA)
                elif eng == 'p':
                    nc.gpsimd.tensor_scalar(dst, src, 1.0, None, ALU.is_ge)
                else:
                    nc.vector.tensor_scalar(dst, src, 1.0, None, ALU.is_ge)

            rb_inA = dram_p.tile([128, 1024], f16)
            rb_outA = dram_p.tile([4, 128, 1024], f16)
            rb_inB = dram_p.tile([128, 1024], f16)
            rb_outB = dram_p.tile([4, 128, 1024], f16)

            def exchange_half(half):
                rb_i, rb_o = (rb_inA, rb_outA) if half == 0 else (rb_inB, rb_outB)
                r_loc_v = r_loc[:].rearrange("p (a w e) -> p a w e", a=8, w=8, e=32)
                nc.sync.dma_start(
                    rb_i[:].rearrange("p (a w e) -> p a w e", a=8, w=4, e=32),
                    r_loc_v[:, :, half * 4:(half + 1) * 4, :])
                if sim_mode:
                    for rk in range(4):
                        nc.sync.dma_start(rb_o[rk], rb_i[:])
                else:
                    nc.gpsimd.collective_compute(
                        "AllGather",
                        mybir.AluOpType.bypass,
                        replica_groups=[[0, 1, 2, 3], [4, 5, 6, 7]],
                        ins=[rb_i[:].opt()],
                        outs=[rb_o[:].opt()],
                    )
                r_all_vv = r_all[:].rearrange("p (a t j e) -> p a t j e", a=2, t=4, j=32, e=32)
                for rk in range(4):
                    srcv = rb_o[rk].rearrange("p (a w e) -> p a w e", a=8, w=4, e=32)                         .rearrange("p (s t) w e -> p s t w e", s=2, t=4)
                    nc.sync.dma_start(
                        r_all_vv[:, :, :, rk * 8 + half * 4: rk * 8 + (half + 1) * 4, :],
                        srcv)

            # ================= phase 1: qkv + LIF + R =================
            ph1 = contextlib.ExitStack()
            ps_a = ph1.enter_context(tc.tile_pool(name="psA", bufs=2, space="PSUM"))
            ps_q = ph1.enter_context(tc.tile_pool(name="psQ", bufs=2, space="PSUM"))
            ps_r = ph1.enter_context(tc.tile_pool(name="psr", bufs=2, space="PSUM"))
            x_tiles = []
            for wi in range(NW):
                x_sb = xin_p.tile([128, 2048], f16, tag="xsb")
                x_tiles.append(x_sb)
                for half in range(2):
                    for kc in range(2):
                        nc.sync.dma_start(
                            x_sb[:, kc * 1024 + half * 512: kc * 1024 + (half + 1) * 512],
                            x_in[wi, kc, :, half * 512:(half + 1) * 512])

                if wi == 0:
                    nc.sync.dma_start(wkv_sb[:, 512:1024], wkv_in[1])
                    nc.sync.dma_start(bkvh_c[:], bkvh_in[:])
                elif wi == 1:
                    load_late_consts()
                skv = skv_p.tile([128, 4096], f16, tag="skv")
                v16h = st_p.tile([128, 1024], f16, tag="vkv")   # kv v/2 state
                for t in range(T):
                    # ---- kv matmuls (token-major); bias via ones-row mm ----
                    ps = ps_a.tile([128, 1024], f32, tag="psa")
                    for sh in range(2):
                        reg = ps[:, sh * 512:(sh + 1) * 512]
                        for kc in range(2):
                            nc.tensor.matmul(
                                reg,
                                lhsT=x_sb[:, kc * 1024 + t * 256 + sh * 128:
                                          kc * 1024 + t * 256 + (sh + 1) * 128],
                                rhs=wkv_sb[:, kc * 512:(kc + 1) * 512],
                                start=(kc == 0), stop=False)
                        nc.tensor.matmul(
                            reg,
                            lhsT=ones1[:].rearrange("o (k f) -> o k f", k=2, f=256)[
                                :, :, :128],
                            rhs=bkvh_c[:].rearrange("o (k f) -> o k f", k=2, f=1024)[
                                :, :, sh * 512:(sh + 1) * 512],
                            start=False, stop=True,
                            perf_mode=mybir.MatmulPerfMode.DoubleRow)
                    # evacuate h-partial = x@W/2 + b/2 (Act); h = hps + v/2
                    hps = st_p.tile([128, 1024], f16, tag="hps")
                    nc.scalar.activation(hps[:], ps[:], ACT.Copy, bias=0.0, scale=1.0)
                    if t == 0:
                        hkv = hps
                    else:
                        hkv = st_p.tile([128, 1024], f16, tag="hkv")
                        nc.vector.tensor_tensor(hkv[:], hps[:], v16h[:], op=ALU.add)
                    s_sl = skv[:, t * 1024:(t + 1) * 1024]
                    spike(SPK_KV[t], s_sl, hkv[:])
                    if t < T - 1:
                        lt = st_p.tile([128, 1024], f16, tag="lt")
                        nc.vector.tensor_scalar(lt[:], hkv[:], 1.0, 0.5, ALU.is_lt, ALU.mult)
                        nc.vector.tensor_tensor(v16h[:], hkv[:], lt[:], op=ALU.mult)

                # ---- R = k^T v per (t, head): col-tiled 4 heads ----
                for slab in range(2):
                    psr = ps_r.tile([128, 128], f32, tag="psr")
                    for t in range(T):
                        for hl in range(4):
                            h = slab * 4 + hl
                            for sh in range(2):
                                base = t * 1024 + sh * 512
                                nc.tensor.matmul(
                                    psr[32 * hl:32 * (hl + 1), t * 32:(t + 1) * 32],
                                    lhsT=skv[:, base + h * 32: base + (h + 1) * 32],
                                    rhs=skv[:, base + 256 + h * 32: base + 256 + (h + 1) * 32],
                                    start=(sh == 0), stop=(sh == 1),
                                    tile_position=(0, 32 * hl),
                                )
                    dst = r_loc[:].rearrange("p (a w e) -> p a w e", a=8, w=8, e=32)[
                        :, slab * 4:(slab + 1) * 4, wi, :]
                    if REVAC == "a":
                        nc.scalar.activation(dst, psr[:], ACT.Copy, bias=0.0, scale=1.0)
                    elif REVAC == "p":
                        nc.gpsimd.tensor_copy(dst, psr[:])
                    else:
                        nc.vector.tensor_copy(dst, psr[:])
                if wi == 3:
                    exchange_half(0)
                elif wi == 7:
                    exchange_half(1)

            # r_all free layout: (slab2, t4, j32, e32)
            r_all_v = r_all[:].rearrange("p (a t j e) -> p a t j e", a=2, t=4, j=32, e=32)

            # ---- q-pass (feature-major) interleaved with gather emission ----
            def emit_q_window(wi):
                x_sb = x_tiles[wi]
                sqw = sq_w[wi]
                sqw_v = sqw[:].rearrange("p (f t e) -> p f t e", f=2, t=4, e=256)
                vq16h = st_p.tile([128, 512], f16, tag="vq")    # q v/2 state
                for t in range(T):
                    psq = ps_q.tile([128, 512], f32, tag="psq")
                    for fc in range(2):
                        reg = psq[:, fc * 256:(fc + 1) * 256]
                        for kc in range(2):
                            nc.tensor.matmul(
                                reg,
                                lhsT=wq_sb[:, kc * 256 + fc * 128:
                                           kc * 256 + (fc + 1) * 128],
                                rhs=x_sb[:, kc * 1024 + t * 256:
                                         kc * 1024 + (t + 1) * 256],
                                start=(kc == 0), stop=False)
                        nc.tensor.matmul(
                            reg,
                            lhsT=bqh_c[:].rearrange("o (k f) -> o k f", k=2, f=256)[
                                :, :, fc * 128:(fc + 1) * 128],
                            rhs=ones1[:].rearrange("o (k f) -> o k f", k=2, f=256),
                            start=False, stop=True,
                            perf_mode=mybir.MatmulPerfMode.DoubleRow)
                    hqps = st_p.tile([128, 512], f16, tag="hqps")
                    nc.scalar.activation(hqps[:], psq[:], ACT.Copy, bias=0.0, scale=1.0)
                    if t == 0:
                        hq = hqps
                    else:
                        hq = st_p.tile([128, 512], f16, tag="hq")
                        nc.vector.tensor_tensor(hq[:], hqps[:], vq16h[:], op=ALU.add)
                    sq_sl = sqw_v[:, :, t, :]
                    spike(SPK_Q[t], sq_sl, hq[:])
                    if t < T - 1:
                        ltq = st_p.tile([128, 512], f16, tag="ltq")
                        nc.vector.tensor_scalar(ltq[:], hq[:], 1.0, 0.5, ALU.is_lt, ALU.mult)
                        nc.vector.tensor_tensor(vq16h[:], hq[:], ltq[:], op=ALU.mult)

            kvw_v = kv_w_all[:].rearrange("p (w st e) -> p w st e", w=NW, st=8, e=32)
            bd_v = bd_all[:].rearrange("p (w st c) -> p w st c", w=NW, st=8, c=128)
            # kv sums: routed gather baked per core, guarded by If on core id
            pid = None if sim_mode else nc.partition_id()

            def emit_gather(gather_windows):
                for r in range(NCORES):
                    if sim_mode and r != 0:
                        continue
                    b_of = r // 4
                    wg = r % 4
                    with (contextlib.nullcontext() if sim_mode else tc.If(pid == r)):
                        for wl in gather_windows:
                            wglob = wg * 8 + wl
                            idxs = [int(j) for j in routing_idx[b_of, wglob]]
                            dst = kv_w_all[:, wl * 256:(wl + 1) * 256]
                            # tree reduction, levels split across DVE and Pool
                            g1a = gt_p.tile([128, 256], f16, tag="g1a")
                            g1b = gt_p.tile([128, 256], f16, tag="g1b")
                            nc.vector.tensor_tensor(
                                g1a[:], r_all_v[:, :, :, idxs[0], :],
                                r_all_v[:, :, :, idxs[1], :], op=ALU.add)
                            nc.vector.tensor_tensor(
                                g1a[:], g1a[:],
                                r_all_v[:, :, :, idxs[2], :], op=ALU.add)
                            nc.vector.tensor_tensor(
                                g1a[:], g1a[:],
                                r_all_v[:, :, :, idxs[3], :], op=ALU.add)
                            g1c = gt_p.tile([128, 256], f16, tag="g1c")
                            nc.gpsimd.tensor_tensor(
                                g1b[:], r_all_v[:, :, :, idxs[4], :],
                                r_all_v[:, :, :, idxs[5], :], op=ALU.add)
                            nc.gpsimd.tensor_tensor(
                                g1c[:], r_all_v[:, :, :, idxs[6], :],
                                r_all_v[:, :, :, idxs[7], :], op=ALU.add)
                            nc.vector.tensor_tensor(g1a[:], g1a[:], g1b[:], op=ALU.add)
                            nc.vector.tensor_tensor(dst, g1a[:], g1c[:], op=ALU.add)

            for wi in range(NW):
                emit_q_window(wi)
                if wi % 2 == 1:
                    emit_gather([wi - 1, wi])
                if wi in (3, 7):
                    hb = wi // 4
                    for hl in range(4):
                        nc.sync.dma_start(
                            bd_v[32 * hl:32 * (hl + 1), hb * 4:(hb + 1) * 4, :,
                                 32 * hl:32 * (hl + 1)],
                            kvw_v[32 * hl:32 * (hl + 1), hb * 4:(hb + 1) * 4])
            ph1.close()
            ph2 = contextlib.ExitStack()
            ps_at = ph2.enter_context(tc.tile_pool(name="psat", bufs=4, space="PSUM"))
            ps_pj = ph2.enter_context(tc.tile_pool(name="pspj", bufs=4, space="PSUM"))


            # ============ segment 2: attn/proj interleaved (lag 2) ===========
            attn_tiles = {}

            def emit_attn(wi):
                bd = bd_all[:, wi * 1024:(wi + 1) * 1024]
                sqw = sq_w[wi]
                attn = xin_p.tile([128, 2 * NTOK], f16, tag="xsb")
                attn_tiles[wi] = attn
                for th in range(2):
                    for slab in range(2):
                        psa = ps_at.tile([128, 512], f32, tag="psat")
                        for tl in range(2):
                            t = th * 2 + tl
                            st = slab * 4 + t
                            nc.tensor.matmul(
                                psa[:, tl * 256:(tl + 1) * 256],
                                lhsT=bd[:, st * 128:(st + 1) * 128],
                                rhs=sqw[:, slab * NTOK + t * 256:
                                        slab * NTOK + (t + 1) * 256],
                                start=True, stop=True,
                            )
                        dst_at = attn[:, slab * NTOK + th * 512:
                                      slab * NTOK + (th + 1) * 512]
                        ev = ATTN_EVAC[(wi * 4 + th * 2 + slab) % len(ATTN_EVAC)]
                        if ev == "a":
                            nc.scalar.activation(dst_at, psa[:], ACT.Copy,
                                                 bias=0.0, scale=1.0)
                        elif ev == "p":
                            nc.gpsimd.tensor_copy(dst_at, psa[:])
                        else:
                            nc.vector.tensor_copy(dst_at, psa[:])

            def emit_proj(wi):
                attn = attn_tiles[wi]
                outsb = out_p.tile([128, 2 * NTOK], f32, tag="outsb")
                outd_v = out_d[wi].rearrange("c p (n f) -> c p n f", n=2, f=512)
                for cft in range(2):
                    for nch in range(2):
                        psp = ps_pj.tile([128, 512], f32, tag="pspj")
                        for kc in range(2):
                            nc.tensor.matmul(
                                psp[:],
                                lhsT=wp_sb[:, kc * 256 + cft * 128: kc * 256 + (cft + 1) * 128],
                                rhs=attn[:, kc * NTOK + nch * 512: kc * NTOK + (nch + 1) * 512],
                                start=(kc == 0), stop=(kc == 1),
                            )
                        nc.scalar.activation(
                            outsb[:, cft * NTOK + nch * 512: cft * NTOK + (nch + 1) * 512],
                            psp[:], ACT.Identity, bias=bp_sb[:, cft:cft + 1], scale=1.0)
                        nc.sync.dma_start(
                            outd_v[cft, :, nch, :],
                            outsb[:, cft * NTOK + nch * 512: cft * NTOK + (nch + 1) * 512])

            for wi in range(NW):
                emit_attn(wi)
                if wi >= 2:
                    emit_proj(wi - 2)
            emit_proj(NW - 2)
            emit_proj(NW - 1)
            ph2.close()

    _split_sync_waits(nc, mybir, maxw=1)
    return nc


def _host_prepost(x, w_qkv, b_qkv):
    """Window partition, routing."""
    xw = x.reshape(T, B, WT, GT, WH, GH, WW, GW, C) \
          .transpose(0, 1, 2, 4, 6, 3, 5, 7, 8).reshape(T, B, W, S, C)
    xbar = xw.mean(axis=(0, 3))                      # [B, W, C]
    q_reg = xbar @ w_qkv[:, :C] + b_qkv[:C]
    k_reg = xbar @ w_qkv[:, C:2 * C] + b_qkv[C:2 * C]
    a_r = np.einsum('bwc,bvc->bwv', q_reg, k_reg)
    routing_idx = np.argsort(-a_r, axis=-1)[:, :, :TOPK]   # [B, W, TOPK]
    return xw, routing_idx


def kernel(x, w_qkv, b_qkv, w_proj, b_proj):
    x = np.ascontiguousarray(np.asarray(x, dtype=np.float32))
    w_qkv = np.asarray(w_qkv, dtype=np.float32)
    b_qkv = np.asarray(b_qkv, dtype=np.float32)
    w_proj = np.asarray(w_proj, dtype=np.float32)
    b_proj = np.asarray(b_proj, dtype=np.float32)

    xw, routing_idx = _host_prepost(x, w_qkv, b_qkv)

    key = (routing_idx.tobytes(), SPK_KV, SPK_Q, ATTN_EVAC, REVAC)
    if key not in _prog_cache:
        _prog_cache.clear()
        _prog_cache[key] = _build_program(routing_idx)
    nc = _prog_cache[key]

    # weights (shared across cores), pre-halved for the LIF /TAU
    wkv_arr = (0.5 * w_qkv[:, C:]).reshape(2, 128, 512).astype(np.float16)
    wq_arr = (0.5 * w_qkv[:, :C]).reshape(2, 128, 256).astype(np.float16)
    import ml_dtypes
    f8 = ml_dtypes.float8_e4m3fn
    bkvh_row = np.tile((0.25 * b_qkv[C:]).astype(f8), 2).reshape(1024)
    bkvh_arr = np.concatenate([bkvh_row, bkvh_row]).reshape(1, 2048)  # kt-major
    bqh_row = (0.25 * b_qkv[:C]).astype(f8).reshape(256)
    bqh_arr = np.concatenate([bqh_row, bqh_row]).reshape(1, 512)      # kt-major
    wp = (SCALE * w_proj).reshape(2, 128, 256).astype(np.float16)
    bp = b_proj.reshape(2, 128, 1).astype(np.float32)

    in_maps = []
    for r in range(NCORES):
        b_of, wg = r // 4, r % 4
        xwc = xw[:, b_of, wg * 8:(wg + 1) * 8]              # [T, 8, S, C]
        xl = np.ascontiguousarray(
            xwc.transpose(1, 3, 0, 2).reshape(NW, 2, 128, NTOK)).astype(np.float16)
        in_maps.append({
            "x_in": xl,
            "wkv_in": wkv_arr, "wq_in": wq_arr,
            "bkvh_in": bkvh_arr, "bqh_in": bqh_arr,
            "wp_in": wp, "bp_in": bp,
        })

    from concourse.bass_utils import run_bass_kernel_spmd
    res = run_bass_kernel_spmd(nc, in_maps, core_ids=list(range(NCORES)))

    # assemble output
    yw = np.empty((T, B, W, S, C), dtype=np.float32)
    for r in range(NCORES):
        b_of, wg = r // 4, r % 4
        o = res.results[r]["out_d"]                          # [NW, 2, 128, NTOK]
        o = o.reshape(NW, 2, 128, T, S).transpose(0, 3, 4, 1, 2).reshape(NW, T, S, C)
        for wl in range(NW):
            yw[:, b_of, wg * 8 + wl] = o[wl]

    y = yw.reshape(T, B, WT, WH, WW, GT, GH, GW, C) \
          .transpose(0, 1, 2, 5, 3, 6, 4, 7, 8).reshape(T, B, Lt, Lh, Lw, C)
    return y
